# revision 1
# baseline (speedup 1.0000x reference)
"""Trainium2 Bass kernel for the attention-LSTM decoder.

Computation (per batch b, all T positions share the same (h0, c0) state):
  h0 = tanh(eh @ bridge_hW.T + hb);  c0 = tanh(ec @ bridge_cW.T + cb)
  pk = enc @ key_W.T;  energy = tanh(pk + (h0 @ query_W.T))
  scores = energy . energy_W;  alphas = softmax(mask(scores))
  ctx = alphas @ enc
  gates = emb[tok] @ W_ih[:, :E].T + [ctx @ W_ih[:, E:].T + h0 @ W_hh.T + b]
  c = sig(f)*c0 + sig(i)*tanh(g);  h = sig(o)*tanh(c)
  out = emb[tok] @ pre_W[:, :E].T + h @ pre_W[:, E:E+H].T + ctx @ pre_W[:, E+H:].T

Sharding: data-parallel over batch B=128 across 8 cores (16 batches each).
Activations are kept feature-major ("transposed") on chip so every matmul
contraction sits on the partition dim; the final projection emits token-major
output directly. The token/attention datapath runs in fp16 (full-rate PE,
half DMA, 2x DVE modes); the per-batch setup path stays fp32.
"""

import numpy as np
from contextlib import ExitStack

import concourse.bass as bass
import concourse.mybir as mybir
import concourse.tile as tile
from concourse import bacc
from concourse.bass_utils import run_bass_kernel_spmd
from concourse.masks import make_identity

FP32 = mybir.dt.float32
F16 = mybir.dt.float16
I32 = mybir.dt.int32
AF = mybir.ActivationFunctionType
OP = mybir.AluOpType
AX = mybir.AxisListType

P = 128
H = 512
E = 256
TWOH = 1024
FOURH = 2048
S = 256
T = 256
V = 10000
N_CORES = 8
B_FULL = 128


def _load_chunked(nc, dst_tile, src_dram, k_chunks, n):
    """DRAM [k_chunks*128, n] -> SBUF [128, k_chunks*n] (chunk-major)."""
    src = src_dram[:].rearrange("(k p) n -> p k n", p=P)
    dst = dst_tile[:].rearrange("p (k n) -> p k n", k=k_chunks)
    nc.sync.dma_start(out=dst, in_=src)


def build_kernel(nc, NB, debug=False, iters=1, phases="ab"):
    """Build the per-core program. NB = batches per core (must be even).

    iters>1 wraps the whole body in a dynamic loop (for timing runs)."""
    ntok = NB * T
    n_ttiles = ntok // 512  # 512-token tiles (2 batches each)

    dbg = {}
    if debug:
        for name, shape in dict(
            d_h0T=[P, 4 * NB], d_c0T=[P, 4 * NB], d_qprojT=[P, 4 * NB],
            d_ctxT=[P, 8 * NB], d_e=[NB, S], d_gc=[P, 16 * NB],
            d_oc=[NB, H], d_ge=[n_ttiles * P, 4 * E],
            d_embT=[n_ttiles * P, 2 * 512], d_hT=[n_ttiles * P, 4 * 512],
            d_encT=[NB * P, 8 * S], d_energy=[NB * P, 4 * S],
        ).items():
            dty = F16 if name in ("d_ge", "d_embT", "d_hT", "d_encT",
                                  "d_energy", "d_e", "d_oc") else FP32
            dbg[name] = nc.dram_tensor(name, shape, dty, kind="ExternalOutput")

    dt = lambda name, shape, dtype=FP32: nc.dram_tensor(
        name, shape, dtype, kind="ExternalInput"
    )

    enc_d = dt("enc", [NB * S, TWOH], F16)
    encT_d = dt("encT", [NB * TWOH, S], F16)
    ehT_d = dt("ehT", [TWOH, NB])
    ecT_d = dt("ecT", [TWOH, NB])
    idx_d = dt("idx", [P, ntok // P], I32)
    mask_d = dt("mask", [1, NB * S])
    maskoff_d = dt("maskoff", [1, NB * S])
    emb_d = dt("emb", [V, E], F16)
    keyWT_d = dt("keyWT", [TWOH, H], F16)
    queryWT_d = dt("queryWT", [H, H])
    energyW_d = dt("energyW", [P, 4], F16)
    wihTemb_d = dt("wihTemb", [E, FOURH], F16)
    wihTctx_d = dt("wihTctx", [TWOH, FOURH], F16)
    whhT_d = dt("whhT", [H, FOURH], F16)
    biasg_d = dt("biasg", [P, 16])
    bhWT_d = dt("bhWT", [TWOH, H])
    bcWT_d = dt("bcWT", [TWOH, H])
    hb_d = dt("hb", [P, 4])
    cb_d = dt("cb", [P, 4])
    preWTemb_d = dt("preWTemb", [E, H], F16)
    preWTh_d = dt("preWTh", [H, H], F16)
    preWTctx_d = dt("preWTctx", [TWOH, H], F16)
    out_d = nc.dram_tensor("out", [ntok, H], FP32, kind="ExternalOutput")

    with ExitStack() as ctx:
        tc = ctx.enter_context(tile.TileContext(nc))
        if iters > 1:
            ctx.enter_context(tc.For_i(0, iters, 1))

        # ---------- constants + small resident tensors ----------
        const = ctx.enter_context(tc.tile_pool(name="const", bufs=1))
        identity_h = const.tile([P, P], F16)
        make_identity(nc, identity_h[:])
        ones_f = const.tile([P, 1], FP32)
        nc.vector.memset(ones_f[:], 1.0)
        ones_h = const.tile([P, 1], F16)
        nc.vector.tensor_copy(ones_h[:], ones_f[:])

        idx_sb = const.tile([P, ntok // P], I32)
        nc.sync.dma_start(out=idx_sb[:], in_=idx_d[:])
        mask_sb = const.tile([1, NB * S], FP32)
        nc.sync.dma_start(out=mask_sb[:], in_=mask_d[:])
        maskoff_sb = const.tile([1, NB * S], FP32)
        nc.sync.dma_start(out=maskoff_sb[:], in_=maskoff_d[:])
        energyW_sb = const.tile([P, 4], F16)
        nc.sync.dma_start(out=energyW_sb[:], in_=energyW_d[:])
        biasg_sb = const.tile([P, 16], FP32)
        nc.sync.dma_start(out=biasg_sb[:], in_=biasg_d[:])
        hb_sb = const.tile([P, 4], FP32)
        nc.sync.dma_start(out=hb_sb[:], in_=hb_d[:])
        cb_sb = const.tile([P, 4], FP32)
        nc.sync.dma_start(out=cb_sb[:], in_=cb_d[:])
        ehT_sb = const.tile([P, 8 * NB], FP32)
        _load_chunked(nc, ehT_sb, ehT_d, 8, NB)
        ecT_sb = const.tile([P, 8 * NB], FP32)
        _load_chunked(nc, ecT_sb, ecT_d, 8, NB)

        # ---------- big resident weights (fp16) ----------
        resident = ctx.enter_context(tc.tile_pool(name="resident", bufs=1))
        keyWT_sb = resident.tile([P, 8 * H], F16)
        _load_chunked(nc, keyWT_sb, keyWT_d, 8, H)
        wihTemb_sb = resident.tile([P, 2 * FOURH], F16)
        _load_chunked(nc, wihTemb_sb, wihTemb_d, 2, FOURH)
        preWTemb_sb = resident.tile([P, 2 * H], F16)
        _load_chunked(nc, preWTemb_sb, preWTemb_d, 2, H)
        preWTh_sb = resident.tile([P, 4 * H], F16)
        _load_chunked(nc, preWTh_sb, preWTh_d, 4, H)
        whh_sb = resident.tile([P, 4 * FOURH], F16)
        _load_chunked(nc, whh_sb, whhT_d, 4, FOURH)
        wcx_sb = resident.tile([P, 8 * FOURH], F16)
        _load_chunked(nc, wcx_sb, wihTctx_d, 8, FOURH)
        pwc_sb = resident.tile([P, 8 * H], F16)
        _load_chunked(nc, pwc_sb, preWTctx_d, 8, H)

        # per-batch state tiles (live through the whole kernel)
        state = ctx.enter_context(tc.tile_pool(name="state", bufs=1))
        h0T_sb = state.tile([P, 4 * NB], FP32)
        c0T_sb = state.tile([P, 4 * NB], FP32)
        qprojT_sb = state.tile([P, 4 * NB], FP32)
        ctxT_sb = state.tile([P, 8 * NB], FP32)
        h0T_h = state.tile([P, 4 * NB], F16)
        ctxT_h = state.tile([P, 8 * NB], F16)
        gc_sb = state.tile([P, 16 * NB], FP32)   # per-(gate-chunk, b) bias
        oc_sb = state.tile([NB, H], F16)         # ctx @ preW_ctx.T

        # ---------- setup: bridge h0/c0, qproj (fp32) ----------
        with tc.tile_pool(name="setup_w", bufs=1) as swp, \
             tc.tile_pool(name="setup_ps", bufs=2, space="PSUM") as sps:
            bhWT_sb = swp.tile([P, 8 * H], FP32, tag="bridge")
            _load_chunked(nc, bhWT_sb, bhWT_d, 8, H)
            for m in range(4):
                ps = sps.tile([P, NB], FP32, tag="ps")
                for k in range(8):
                    nc.tensor.matmul(
                        ps[:], bhWT_sb[:, k * H + m * P:k * H + m * P + P],
                        ehT_sb[:, k * NB:(k + 1) * NB],
                        start=(k == 0), stop=(k == 7))
                nc.scalar.activation(h0T_sb[:, m * NB:(m + 1) * NB], ps[:],
                                     AF.Tanh, bias=hb_sb[:, m:m + 1])
            bcWT_sb = swp.tile([P, 8 * H], FP32, tag="bridge")
            _load_chunked(nc, bcWT_sb, bcWT_d, 8, H)
            for m in range(4):
                ps = sps.tile([P, NB], FP32, tag="ps")
                for k in range(8):
                    nc.tensor.matmul(
                        ps[:], bcWT_sb[:, k * H + m * P:k * H + m * P + P],
                        ecT_sb[:, k * NB:(k + 1) * NB],
                        start=(k == 0), stop=(k == 7))
                nc.scalar.activation(c0T_sb[:, m * NB:(m + 1) * NB], ps[:],
                                     AF.Tanh, bias=cb_sb[:, m:m + 1])
            qWT_sb = swp.tile([P, 4 * H], FP32, tag="bridge")
            _load_chunked(nc, qWT_sb, queryWT_d, 4, H)
            for m in range(4):
                ps = sps.tile([P, NB], FP32, tag="ps")
                for k in range(4):
                    nc.tensor.matmul(
                        ps[:], qWT_sb[:, k * H + m * P:k * H + m * P + P],
                        h0T_sb[:, k * NB:(k + 1) * NB],
                        start=(k == 0), stop=(k == 3))
                nc.vector.tensor_copy(qprojT_sb[:, m * NB:(m + 1) * NB], ps[:])
            nc.vector.tensor_copy(h0T_h[:], h0T_sb[:])

        if debug:
            nc.sync.dma_start(out=dbg["d_h0T"][:], in_=h0T_sb[:])
            nc.sync.dma_start(out=dbg["d_c0T"][:], in_=c0T_sb[:])
            nc.sync.dma_start(out=dbg["d_qprojT"][:], in_=qprojT_sb[:])

        # ---------- phase A: attention (per batch) ----------
        with tc.tile_pool(name="encp", bufs=3) as encp, \
             tc.tile_pool(name="encTp", bufs=2) as encTp, \
             tc.tile_pool(name="enerp", bufs=2) as enerp, \
             tc.tile_pool(name="arow", bufs=4) as arow, \
             tc.tile_pool(name="ps_pk", bufs=2, space="PSUM") as ps_pk, \
             tc.tile_pool(name="ps_sc", bufs=1, space="PSUM") as ps_sc, \
             tc.tile_pool(name="ps_sm", bufs=2, space="PSUM") as ps_sm, \
             tc.tile_pool(name="ps_cu", bufs=1, space="PSUM") as ps_cu:
            if "a" not in phases:
                nc.vector.memset(ctxT_sb[:], 0.1)
            prev = None
            for bb in range(NB + 1 if "a" in phases else 0):
                if bb < NB:
                    b = bb
                    enc_t = encp.tile([P, 2 * TWOH], F16, tag="enc")
                    nc.sync.dma_start(
                        out=enc_t[:].rearrange("p (c d) -> p c d", c=2),
                        in_=enc_d[b * S:(b + 1) * S, :].rearrange(
                            "(c p) d -> p c d", p=P))
                    # encT[dchunk k][:, s] = enc[s, k*128:...] (host-transposed)
                    encT_t = encTp.tile([P, 8 * S], F16, tag="encT")
                    nc.sync.dma_start(
                        out=encT_t[:].rearrange("p (k s) -> p k s", k=8),
                        in_=encT_d[b * TWOH:(b + 1) * TWOH, :].rearrange(
                            "(k p) s -> p k s", p=P))
                    if debug:
                        nc.sync.dma_start(
                            out=dbg["d_encT"][b * P:(b + 1) * P, :],
                            in_=encT_t[:])
                    # energy = tanh(keyW.T-proj + qproj_b)
                    ener_t = enerp.tile([P, 4 * S], F16, tag="ener")
                    for m in range(4):
                        pk = ps_pk.tile([P, S], FP32, tag="pk")
                        for k in range(8):
                            nc.tensor.matmul(
                                pk[:],
                                keyWT_sb[:, k * H + m * P:k * H + m * P + P],
                                encT_t[:, k * S:(k + 1) * S],
                                start=(k == 0), stop=(k == 7))
                        nc.scalar.activation(
                            ener_t[:, m * S:(m + 1) * S], pk[:], AF.Tanh,
                            bias=qprojT_sb[:, m * NB + b:m * NB + b + 1])
                    if debug:
                        nc.sync.dma_start(
                            out=dbg["d_energy"][b * P:(b + 1) * P, :],
                            in_=ener_t[:])
                    # scores (1 x S), masked, stabilized, exponentiated
                    sc = ps_sc.tile([1, S], FP32, tag="sc")
                    for m in range(4):
                        nc.tensor.matmul(sc[:], energyW_sb[:, m:m + 1],
                                         ener_t[:, m * S:(m + 1) * S],
                                         start=(m == 0), stop=(m == 3))
                    sm_t = arow.tile([1, S], FP32, tag="sm")
                    nc.vector.tensor_tensor(
                        out=sm_t[:], in0=sc[:],
                        in1=mask_sb[0:1, b * S:(b + 1) * S], op=OP.mult)
                    nc.vector.tensor_tensor(
                        out=sm_t[:], in0=sm_t[:],
                        in1=maskoff_sb[0:1, b * S:(b + 1) * S], op=OP.add)
                    nmx = arow.tile([1, 1], FP32, tag="nmx")
                    nc.vector.tensor_reduce(nmx[:], sm_t[:], AX.X, OP.max,
                                            negate=True)
                    e_t = arow.tile([1, S], F16, tag="e")
                    nc.scalar.activation(e_t[:], sm_t[:], AF.Exp,
                                         bias=nmx[0:1, 0:1])
                    if debug:
                        nc.sync.dma_start(out=dbg["d_e"][b:b + 1, :],
                                          in_=e_t[:])
                    cur = (b, enc_t, e_t)
                else:
                    cur = None
                if prev is not None:
                    b, enc_t, e_t = prev
                    # eT (S x 1) via K=1 matmuls
                    eT_t = arow.tile([P, 2], F16, tag="eT")
                    for c in range(2):
                        tp1 = ps_sm.tile([P, 1], FP32, tag="small")
                        nc.tensor.matmul(tp1[:], e_t[0:1, c * P:(c + 1) * P],
                                         ones_h[0:1, 0:1], start=True,
                                         stop=True)
                        nc.vector.tensor_copy(eT_t[:, c:c + 1], tp1[:])
                    # Z = sum(e)
                    zps = ps_sm.tile([1, 1], FP32, tag="small")
                    for c in range(2):
                        nc.tensor.matmul(zps[:], eT_t[:, c:c + 1],
                                         ones_h[:, 0:1],
                                         start=(c == 0), stop=(c == 1))
                    # ctx_u = e @ enc  (1 x 2H)
                    cu = ps_cu.tile([1, TWOH], FP32, tag="cu")
                    for c in range(2):
                        for n in range(2):
                            nc.tensor.matmul(
                                cu[0:1, n * H:(n + 1) * H], eT_t[:, c:c + 1],
                                enc_t[:, c * TWOH + n * H:
                                      c * TWOH + (n + 1) * H],
                                start=(c == 0), stop=(c == 1))
                    rz = arow.tile([1, 1], FP32, tag="rz")
                    nc.vector.reciprocal(rz[:], zps[:])
                    ctxr_t = arow.tile([1, TWOH], FP32, tag="ctxr")
                    nc.vector.tensor_scalar_mul(ctxr_t[:], cu[:], rz[0:1, 0:1])
                    # scatter ctx into ctxT (2H x NB) via K=1 matmuls
                    for k in range(8):
                        tpc = ps_sm.tile([P, 1], FP32, tag="small")
                        nc.tensor.matmul(tpc[:],
                                         ctxr_t[0:1, k * P:(k + 1) * P],
                                         ones_f[0:1, 0:1], start=True,
                                         stop=True)
                        nc.vector.tensor_copy(
                            ctxT_sb[:, k * NB + b:k * NB + b + 1], tpc[:])
                prev = cur

        # ---------- phase A2: per-batch gate constants + out-ctx ----------
        with tc.tile_pool(name="ps_a2", bufs=1, space="PSUM") as ps_a2:
            nc.vector.tensor_copy(ctxT_h[:], ctxT_sb[:])
            gcps = ps_a2.tile([P, 16 * NB], FP32, tag="gc")
            for m in range(16):
                for k in range(4):
                    nc.tensor.matmul(
                        gcps[:, m * NB:(m + 1) * NB],
                        whh_sb[:, k * FOURH + m * P:k * FOURH + m * P + P],
                        h0T_h[:, k * NB:(k + 1) * NB],
                        start=(k == 0), stop=False)
                for k in range(8):
                    nc.tensor.matmul(
                        gcps[:, m * NB:(m + 1) * NB],
                        wcx_sb[:, k * FOURH + m * P:k * FOURH + m * P + P],
                        ctxT_h[:, k * NB:(k + 1) * NB],
                        start=False, stop=(k == 7))
            biasg_bc = bass.AP(
                biasg_sb[:].tensor, biasg_sb[:].offset,
                [biasg_sb[:].ap[0], biasg_sb[:].ap[1], [0, NB]])
            nc.vector.tensor_tensor(
                out=gc_sb[:].rearrange("p (m b) -> p m b", m=16),
                in0=gcps[:].rearrange("p (m b) -> p m b", m=16),
                in1=biasg_bc, op=OP.add)
            ocps = ps_a2.tile([NB, H], FP32, tag="oc")
            for k in range(8):
                nc.tensor.matmul(ocps[:], ctxT_h[:, k * NB:(k + 1) * NB],
                                 pwc_sb[:, k * H:(k + 1) * H],
                                 start=(k == 0), stop=(k == 7))
            nc.vector.tensor_copy(oc_sb[:], ocps[:])

        if debug:
            nc.sync.dma_start(out=dbg["d_ctxT"][:], in_=ctxT_sb[:])
            nc.sync.dma_start(out=dbg["d_gc"][:], in_=gc_sb[:])
            nc.sync.dma_start(out=dbg["d_oc"][:], in_=oc_sb[:])

        # ---------- phase B: gather + LSTM gates + output projection ----------
        with tc.tile_pool(name="gep", bufs=2) as gep, \
             tc.tile_pool(name="embTp", bufs=3) as embTp, \
             tc.tile_pool(name="hTp", bufs=3) as hTp, \
             tc.tile_pool(name="lstm", bufs=2) as lstm, \
             tc.tile_pool(name="outp", bufs=3) as outp, \
             tc.tile_pool(name="ps_tpB", bufs=2, space="PSUM") as ps_tpB, \
             tc.tile_pool(name="ps_g", bufs=4, space="PSUM") as ps_g, \
             tc.tile_pool(name="ps_o", bufs=2, space="PSUM") as ps_o:
            prevB = None
            for tt in range(n_ttiles + 1 if "b" in phases else 0):
                if tt < n_ttiles:
                    t = tt
                    b0 = 2 * t
                    ge_t = gep.tile([P, 4 * E], F16, tag="ge")
                    for j in range(4):
                        nc.gpsimd.indirect_dma_start(
                            out=ge_t[:, j * E:(j + 1) * E], out_offset=None,
                            in_=emb_d[:],
                            in_offset=bass.IndirectOffsetOnAxis(
                                ap=idx_sb[:, t * 4 + j:t * 4 + j + 1], axis=0))
                    embT_t = embTp.tile([P, 2 * 512], F16, tag="embT")
                    for j in range(4):
                        for e in range(2):
                            tp = ps_tpB.tile([P, P], F16, tag="tpB")
                            nc.tensor.transpose(
                                tp[:],
                                ge_t[:, j * E + e * P:j * E + (e + 1) * P],
                                identity_h[:])
                            nc.vector.tensor_copy(
                                embT_t[:, e * 512 + j * P:
                                       e * 512 + (j + 1) * P], tp[:])
                    if debug:
                        nc.sync.dma_start(out=dbg["d_ge"][t * P:(t + 1) * P, :],
                                          in_=ge_t[:])
                        nc.sync.dma_start(
                            out=dbg["d_embT"][t * P:(t + 1) * P, :],
                            in_=embT_t[:])
                    hT_t = hTp.tile([P, 4 * 512], F16, tag="hT")
                    for hs in range(4):
                        psg = []
                        for g in range(4):
                            mg = g * 4 + hs
                            pg = ps_g.tile([P, 512], FP32, tag="pg")
                            for k in range(2):
                                nc.tensor.matmul(
                                    pg[:],
                                    wihTemb_sb[:, k * FOURH + mg * P:
                                               k * FOURH + mg * P + P],
                                    embT_t[:, k * 512:(k + 1) * 512],
                                    start=(k == 0), stop=(k == 1))
                            psg.append(pg)
                        sI = lstm.tile([P, 512], F16, tag="sI")
                        sF = lstm.tile([P, 512], F16, tag="sF")
                        tG = lstm.tile([P, 512], F16, tag="tG")
                        sO = lstm.tile([P, 512], F16, tag="sO")
                        for x in range(2):
                            b = b0 + x
                            cs = slice(x * S, (x + 1) * S)
                            for pg, dst, fn, g in ((psg[0], sI, AF.Sigmoid, 0),
                                                   (psg[1], sF, AF.Sigmoid, 1),
                                                   (psg[2], tG, AF.Tanh, 2),
                                                   (psg[3], sO, AF.Sigmoid, 3)):
                                mg = g * 4 + hs
                                nc.scalar.activation(
                                    dst[:, cs], pg[:, cs], fn,
                                    bias=gc_sb[:, mg * NB + b:mg * NB + b + 1])
                        tmp = lstm.tile([P, 512], F16, tag="tmp")
                        nc.vector.tensor_tensor(out=tmp[:], in0=sI[:],
                                                in1=tG[:], op=OP.mult)
                        cc = lstm.tile([P, 512], F16, tag="cc")
                        for x in range(2):
                            b = b0 + x
                            cs = slice(x * S, (x + 1) * S)
                            nc.vector.tensor_scalar_mul(
                                cc[:, cs], sF[:, cs],
                                c0T_sb[:, hs * NB + b:hs * NB + b + 1])
                        nc.vector.tensor_tensor(out=cc[:], in0=cc[:],
                                                in1=tmp[:], op=OP.add)
                        tanC = lstm.tile([P, 512], F16, tag="tanC")
                        nc.scalar.activation(tanC[:], cc[:], AF.Tanh)
                        nc.vector.tensor_tensor(
                            out=hT_t[:, hs * 512:(hs + 1) * 512], in0=sO[:],
                            in1=tanC[:], op=OP.mult)
                    if debug:
                        nc.sync.dma_start(out=dbg["d_hT"][t * P:(t + 1) * P, :],
                                          in_=hT_t[:])
                    curB = (t, b0, embT_t, hT_t)
                else:
                    curB = None
                if prevB is not None:
                    t, b0, embT_t, hT_t = prevB
                    # output projection: out[tok, :] (token-major)
                    for tci in range(4):
                        b = b0 + (0 if tci < 2 else 1)
                        po = ps_o.tile([P, H], FP32, tag="po")
                        for k in range(2):
                            nc.tensor.matmul(
                                po[:], embT_t[:, k * 512 + tci * P:
                                              k * 512 + tci * P + P],
                                preWTemb_sb[:, k * H:(k + 1) * H],
                                start=(k == 0), stop=False)
                        for k in range(4):
                            nc.tensor.matmul(
                                po[:], hT_t[:, k * 512 + tci * P:
                                            k * 512 + tci * P + P],
                                preWTh_sb[:, k * H:(k + 1) * H],
                                start=False, stop=False)
                        nc.tensor.matmul(
                            po[:],
                            identity_h[0:NB, b:b + 1].to_broadcast([NB, P]),
                            oc_sb[:], start=False, stop=True)
                        o_t = outp.tile([P, H], FP32, tag="o")
                        nc.vector.tensor_copy(o_t[:], po[:])
                        nc.sync.dma_start(
                            out=out_d[t * 512 + tci * P:
                                      t * 512 + (tci + 1) * P, :],
                            in_=o_t[:])
                prevB = curB
    return nc


# ---------------------------------------------------------------------------
# host side
# ---------------------------------------------------------------------------

def _chunkmajor(v, chunks, dtype=np.float32):
    """[chunks*128] vector -> [128, chunks] (column k = chunk k)."""
    return np.ascontiguousarray(v.reshape(chunks, P).T).astype(dtype)


def prep_inputs(inputs, n_cores=N_CORES):
    """Shard + lay out the full inputs into per-core input maps."""
    f32 = lambda x: np.asarray(x, dtype=np.float32)
    f16 = lambda x: np.ascontiguousarray(
        np.asarray(x, dtype=np.float32)).astype(np.float16)
    tgt_seq = np.asarray(inputs["tgt_seq"]).astype(np.int32)
    enc = f32(inputs["encoder_output"])
    eh = f32(inputs["encoder_hidden"])[0]
    ec = f32(inputs["encoder_cell"])[0]
    src_pos = np.asarray(inputs["src_pos"])
    W_ih = f32(inputs["W_ih"])
    pre_W = f32(inputs["pre_W"])

    B = tgt_seq.shape[0]
    NB = B // n_cores

    shared = dict(
        emb=f16(inputs["emb"]),
        keyWT=f16(f32(inputs["key_W"]).T),
        queryWT=np.ascontiguousarray(f32(inputs["query_W"]).T),
        energyW=_chunkmajor(f32(inputs["energy_W"])[0], 4, np.float16),
        wihTemb=f16(W_ih[:, :E].T),
        wihTctx=f16(W_ih[:, E:].T),
        whhT=f16(f32(inputs["W_hh"]).T),
        biasg=_chunkmajor(f32(inputs["b_ih"]) + f32(inputs["b_hh"]), 16),
        bhWT=np.ascontiguousarray(f32(inputs["bridge_hW"]).T),
        bcWT=np.ascontiguousarray(f32(inputs["bridge_cW"]).T),
        hb=_chunkmajor(f32(inputs["bridge_hb"]), 4),
        cb=_chunkmajor(f32(inputs["bridge_cb"]), 4),
        preWTemb=f16(pre_W[:, :E].T),
        preWTh=f16(pre_W[:, E:E + H].T),
        preWTctx=f16(pre_W[:, E + H:].T),
    )

    in_maps = []
    for i in range(n_cores):
        sl = slice(i * NB, (i + 1) * NB)
        m = np.ascontiguousarray(
            src_pos[sl, 0, :].astype(np.float32).reshape(1, NB * S))
        idx = tgt_seq[sl].reshape(-1)
        enc16 = enc[sl].astype(np.float16)
        in_maps.append(dict(
            enc=np.ascontiguousarray(enc16.reshape(NB * S, TWOH)),
            encT=np.ascontiguousarray(enc16.transpose(0, 2, 1)).reshape(
                NB * TWOH, S),
            ehT=np.ascontiguousarray(eh[sl].T),
            ecT=np.ascontiguousarray(ec[sl].T),
            idx=np.ascontiguousarray(idx.reshape(-1, P).T),
            mask=m,
            maskoff=np.ascontiguousarray(-1e9 * (1.0 - m)),
            **shared,
        ))
    return in_maps, NB


_CACHED = {}


def _get_nc(NB):
    if NB not in _CACHED:
        nc = bacc.Bacc("TRN2", target_bir_lowering=False, debug=False)
        build_kernel(nc, NB)
        nc.compile()
        _CACHED[NB] = nc
    return _CACHED[NB]


def kernel(**inputs):
    in_maps, NB = prep_inputs(inputs, N_CORES)
    nc = _get_nc(NB)
    res = run_bass_kernel_spmd(nc, in_maps, list(range(N_CORES)))
    B = np.asarray(inputs["tgt_seq"]).shape[0]
    out = np.empty((B, T, H), dtype=np.float32)
    for i in range(N_CORES):
        out[i * NB:(i + 1) * NB] = res.results[i]["out"].reshape(NB, T, H)
    return out



# revision 9
# speedup vs baseline: 1.0478x; 1.0478x over previous
"""Trainium2 Bass kernel for the attention-LSTM decoder (restructured).

Computation (all T positions share (h0, c0); see reference):
  h0 = tanh(eh @ bridge_hW.T);  c0 = tanh(ec @ bridge_cW.T)
  energy = tanh(enc @ key_W.T + h0 @ query_W.T);  scores = energy . eW
  alphas = softmax(mask(scores));  ctx = alphas @ enc
  gates = emb[tok] @ W_ih[:,:E].T + [ctx @ W_ih[:,E:].T + h0 @ W_hh.T + b]
  c = sig(f)*c0 + sig(i)*tanh(g);  h = sig(o)*tanh(c)
  out = emb[tok] @ preW[:,:E].T + h @ preW[:,E:E+H].T + ctx @ preW[:,E+H:].T

Sharding: data-parallel over batch B=128 across 8 cores (NB=16 each).

Key structure vs the naive version:
 - keyW projection loops (m,k) outer / batch inner with a k-major host
   layout so each stationary is loaded once and streams N=512 (2 batches).
 - scores land in a single [16,S] PSUM tile via diag-expanded energy_W
   stationaries; softmax runs once on [16,S] rows (exp uses accum_out).
 - ctx for all batches accumulates into one [16,2H] PSUM tile via
   masked-diagonal alphasT stationaries.
 - gate consts / out consts computed batch-major [16,4H]/[16,H] with
   N=512 matmuls, then PE-transposed to the per-partition layouts B needs.
 - phase B: token embeddings gathered in fp8, gates = one fp8 DoubleRow
   matmul per (hs,gate); bias applied on DVE (col-pair broadcast APs);
   activations batched as [128,1536] sigmoid + [128,512] tanh.
 - output projection accumulates oc + emb(fp8 DR) + h(fp16) in PSUM and
   DMAs straight from PSUM to DRAM.
"""

import numpy as np
import ml_dtypes
from contextlib import ExitStack

import concourse.bass as bass
import concourse.mybir as mybir
import concourse.tile as tile
from concourse import bacc
from concourse.bass_utils import run_bass_kernel_spmd
from concourse.masks import make_identity

FP32 = mybir.dt.float32
F16 = mybir.dt.float16
F8 = mybir.dt.float8e4
I32 = mybir.dt.int32
AF = mybir.ActivationFunctionType
OP = mybir.AluOpType
AX = mybir.AxisListType
PM = mybir.MatmulPerfMode

P = 128
H = 512
E = 256
TWOH = 1024
FOURH = 2048
S = 256
T = 256
V = 10000
N_CORES = 8
B_FULL = 128
NB = 16
NTOK = NB * T          # 4096
NTT = NTOK // 512      # 8 token tiles (512 tokens = 2 batches each)


def _load_chunked(nc, dst_tile, src_dram, k_chunks, n):
    """DRAM [k_chunks*128, n] -> SBUF [128, k_chunks*n] (chunk-major)."""
    src = src_dram[:].rearrange("(k p) n -> p k n", p=P)
    dst = dst_tile[:].rearrange("p (k n) -> p k n", k=k_chunks)
    nc.sync.dma_start(out=dst, in_=src)


def _colpair(t, col0, rep):
    """AP reading cols [col0, col0+1] of tile t, each broadcast rep times."""
    ap = t[:]
    return bass.AP(ap.tensor, ap.offset + col0, [ap.ap[0], [1, 2], [0, rep]])


def _diag_out(t, col0):
    """AP writing 16 cols of tile t at col0 + 17*j (block-diagonal)."""
    ap = t[:]
    return bass.AP(ap.tensor, ap.offset + col0, [ap.ap[0], [17, 16]])


def build_kernel(nc, debug=False):
    dt = lambda name, shape, dtype=FP32: nc.dram_tensor(
        name, shape, dtype, kind="ExternalInput")

    encT_d = dt("encTkm", [P, 8 * NB * S], F16)     # [p,(k b s)] k-major
    enc_d = dt("enc", [NB * S, TWOH], F16)          # S-major per batch
    ehT_d = dt("ehT", [TWOH, NB])
    ecT_d = dt("ecT", [TWOH, NB])
    idx_d = dt("idx", [P, NTOK // P], I32)
    mask_d = dt("mask", [NB, S])
    maskoff_d = dt("maskoff", [NB, S])
    emb_d = dt("emb", [V, E], F8)
    keyWT_d = dt("keyWT", [TWOH, H], F16)
    queryWT_d = dt("queryWT", [H, H])
    eWd_d = dt("eWd", [P, 4 * NB * NB], F16)        # diag-expanded energy_W
    wih8_d = dt("wih8", [E, FOURH], F8)
    whhT_d = dt("whhT", [H, FOURH], F16)
    wcxT_d = dt("wcxT", [TWOH, FOURH], F16)
    biasg_d = dt("biasg", [1, FOURH], F16)
    bhWT_d = dt("bhWT", [TWOH, H])
    bcWT_d = dt("bcWT", [TWOH, H])
    hb_d = dt("hb", [P, 4])
    cb_d = dt("cb", [P, 4])
    preW8_d = dt("preW8", [E, H], F8)
    preWTh_d = dt("preWTh", [H, H], F16)
    preWTc_d = dt("preWTc", [TWOH, H], F16)
    out_d = nc.dram_tensor("out", [NTOK, H], F16, kind="ExternalOutput")

    dbg = {}
    if debug:
        for name, shape, dty in [
            ("d_energy", [P, 4 * NB * S], F16), ("d_alpha", [NB, S], F16),
            ("d_ctx", [NB, TWOH], F16), ("d_gc", [NB, FOURH], F16),
            ("d_oc", [NB, H], F16), ("d_embT", [P, NTT * TWOH], F8),
            ("d_hT", [P, NTT * FOURH], F16),
        ]:
            dbg[name] = nc.dram_tensor(name, shape, dty, kind="ExternalOutput")

    with ExitStack() as ctx:
        tc = ctx.enter_context(tile.TileContext(nc))

        # ---------- constants ----------
        const = ctx.enter_context(tc.tile_pool(name="const", bufs=1))
        identity_h = const.tile([P, P], F16)
        make_identity(nc, identity_h[:])
        identity_8 = const.tile([P, P], F8)
        nc.vector.tensor_copy(identity_8[:], identity_h[:])
        ones16 = const.tile([1, NB], F16)
        nc.vector.memset(ones16[:], 1.0)

        idx_sb = const.tile([P, NTOK // P], I32)
        nc.sync.dma_start(out=idx_sb[:], in_=idx_d[:])
        mask_sb = const.tile([NB, S], FP32)
        nc.sync.dma_start(out=mask_sb[:], in_=mask_d[:])
        maskoff_sb = const.tile([NB, S], FP32)
        nc.sync.dma_start(out=maskoff_sb[:], in_=maskoff_d[:])
        eWd_sb = const.tile([P, 4 * NB * NB], F16)
        nc.sync.dma_start(out=eWd_sb[:], in_=eWd_d[:])
        biasg_sb = const.tile([1, FOURH], F16)
        nc.sync.dma_start(out=biasg_sb[:], in_=biasg_d[:])
        hb_sb = const.tile([P, 4], FP32)
        nc.sync.dma_start(out=hb_sb[:], in_=hb_d[:])
        cb_sb = const.tile([P, 4], FP32)
        nc.sync.dma_start(out=cb_sb[:], in_=cb_d[:])
        ehT_sb = const.tile([P, 8 * NB], FP32)
        _load_chunked(nc, ehT_sb, ehT_d, 8, NB)
        ecT_sb = const.tile([P, 8 * NB], FP32)
        _load_chunked(nc, ecT_sb, ecT_d, 8, NB)

        # ---------- token embedding gather (fp8), issued up front ----------
        gep = ctx.enter_context(tc.tile_pool(name="gep", bufs=1))
        ge_all = gep.tile([P, NTOK // P * E], F8)
        for j in range(NTOK // P):
            nc.gpsimd.indirect_dma_start(
                out=ge_all[:, j * E:(j + 1) * E], out_offset=None,
                in_=emb_d[:],
                in_offset=bass.IndirectOffsetOnAxis(
                    ap=idx_sb[:, j:j + 1], axis=0))

        # ---------- phase-B weights (prefetch; small) ----------
        bw = ctx.enter_context(tc.tile_pool(name="bw", bufs=1))
        wih8_sb = bw.tile([P, 2 * FOURH], F8)
        _load_chunked(nc, wih8_sb, wih8_d, 2, FOURH)
        whh_sb = bw.tile([P, 4 * FOURH], F16)
        _load_chunked(nc, whh_sb, whhT_d, 4, FOURH)
        preW8_sb = bw.tile([P, 2 * H], F8)
        _load_chunked(nc, preW8_sb, preW8_d, 2, H)
        preWTh_sb = bw.tile([P, 4 * H], F16)
        _load_chunked(nc, preWTh_sb, preWTh_d, 4, H)
        pwc_sb = bw.tile([P, 8 * H], F16)
        _load_chunked(nc, pwc_sb, preWTc_d, 8, H)

        # ---------- state ----------
        state = ctx.enter_context(tc.tile_pool(name="state", bufs=1))
        h0T_sb = state.tile([P, 4 * NB], FP32)
        c0T_sb = state.tile([P, 4 * NB], FP32)
        qprojT_sb = state.tile([P, 4 * NB], FP32)
        h0T_h = state.tile([P, 4 * NB], F16)
        alphas_n = state.tile([NB, S], F16)
        amask = state.tile([P, 2 * S], F16)
        ctx_bm = state.tile([NB, TWOH], F16)
        ctxT_sb = state.tile([P, 8 * NB], F16)
        gc_bm = state.tile([NB, FOURH], F16)
        gcT_sb = state.tile([P, 16 * NB], F16)
        oc_sb = state.tile([NB, H], F16)
        zsum = state.tile([NB, 1], FP32)
        rz = state.tile([NB, 1], FP32)
        nmx = state.tile([NB, 1], FP32)

        # ---------- setup: bridge h0/c0, qproj (fp32) ----------
        with tc.tile_pool(name="setup_w", bufs=1) as swp, \
             tc.tile_pool(name="setup_ps", bufs=2, space="PSUM") as sps:
            bhWT_sb = swp.tile([P, 8 * H], FP32, tag="bridge")
            _load_chunked(nc, bhWT_sb, bhWT_d, 8, H)
            for m in range(4):
                ps = sps.tile([P, NB], FP32, tag="ps")
                for k in range(8):
                    nc.tensor.matmul(
                        ps[:], bhWT_sb[:, k * H + m * P:k * H + m * P + P],
                        ehT_sb[:, k * NB:(k + 1) * NB],
                        start=(k == 0), stop=(k == 7))
                nc.scalar.activation(h0T_sb[:, m * NB:(m + 1) * NB], ps[:],
                                     AF.Tanh, bias=hb_sb[:, m:m + 1])
            bcWT_sb = swp.tile([P, 8 * H], FP32, tag="bridge")
            _load_chunked(nc, bcWT_sb, bcWT_d, 8, H)
            for m in range(4):
                ps = sps.tile([P, NB], FP32, tag="ps")
                for k in range(8):
                    nc.tensor.matmul(
                        ps[:], bcWT_sb[:, k * H + m * P:k * H + m * P + P],
                        ecT_sb[:, k * NB:(k + 1) * NB],
                        start=(k == 0), stop=(k == 7))
                nc.scalar.activation(c0T_sb[:, m * NB:(m + 1) * NB], ps[:],
                                     AF.Tanh, bias=cb_sb[:, m:m + 1])
            qWT_sb = swp.tile([P, 4 * H], FP32, tag="bridge")
            _load_chunked(nc, qWT_sb, queryWT_d, 4, H)
            for m in range(4):
                ps = sps.tile([P, NB], FP32, tag="ps")
                for k in range(4):
                    nc.tensor.matmul(
                        ps[:], qWT_sb[:, k * H + m * P:k * H + m * P + P],
                        h0T_sb[:, k * NB:(k + 1) * NB],
                        start=(k == 0), stop=(k == 3))
                nc.vector.tensor_copy(qprojT_sb[:, m * NB:(m + 1) * NB], ps[:])
            nc.vector.tensor_copy(h0T_h[:], h0T_sb[:])

        # ---------- A1: keyW projection -> energy (fp16) ----------
        ea = ctx.enter_context(tc.tile_pool(name="energy", bufs=1))
        energy_all = ea.tile([P, 4 * NB * S], F16)
        with tc.tile_pool(name="kw", bufs=1) as kwp, \
             tc.tile_pool(name="ps_pk", bufs=1, space="PSUM") as ps_pk:
            keyWT_sb = kwp.tile([P, 8 * H], F16)
            _load_chunked(nc, keyWT_sb, keyWT_d, 8, H)
            encT_sb = kwp.tile([P, 8 * NB * S], F16)
            nc.sync.dma_start(out=encT_sb[:], in_=encT_d[:])
            pk_tiles = [ps_pk.tile([P, 2 * S], FP32, tag=f"pk{i}",
                                   name=f"pk{i}") for i in range(8)]
            for m in range(4):
                for k in range(8):
                    stat = keyWT_sb[:, k * H + m * P:k * H + m * P + P]
                    for bp in range(8):
                        nc.tensor.matmul(
                            pk_tiles[bp][:], stat,
                            encT_sb[:, (k * NB + 2 * bp) * S:
                                    (k * NB + 2 * bp + 2) * S],
                            start=(k == 0), stop=(k == 7))
                for b in range(NB):
                    nc.scalar.activation(
                        energy_all[:, (m * NB + b) * S:(m * NB + b + 1) * S],
                        pk_tiles[b // 2][:, (b % 2) * S:(b % 2 + 1) * S],
                        AF.Tanh, bias=qprojT_sb[:, m * NB + b:m * NB + b + 1])
        if debug:
            nc.sync.dma_start(out=dbg["d_energy"][:], in_=energy_all[:])

        # ---------- A2+A3: scores [16,S] + softmax ----------
        with tc.tile_pool(name="smx", bufs=1) as smx, \
             tc.tile_pool(name="ps_sc", bufs=1, space="PSUM") as ps_sc, \
             tc.tile_pool(name="ps_tp", bufs=2, space="PSUM") as ps_tp:
            scps = ps_sc.tile([NB, S], FP32)
            for m in range(4):
                for b in range(NB):
                    nc.tensor.matmul(
                        scps[:],
                        eWd_sb[:, (m * NB + b) * NB:(m * NB + b + 1) * NB],
                        energy_all[:, (m * NB + b) * S:(m * NB + b + 1) * S],
                        start=(m == 0 and b == 0), stop=(m == 3 and b == 15))
            sm = smx.tile([NB, S], FP32)
            nc.vector.tensor_tensor(out=sm[:], in0=scps[:], in1=mask_sb[:],
                                    op=OP.mult)
            nc.vector.tensor_tensor(out=sm[:], in0=sm[:], in1=maskoff_sb[:],
                                    op=OP.add)
            nc.vector.tensor_reduce(nmx[:], sm[:], AX.X, OP.max, negate=True)
            eu = smx.tile([NB, S], F16)
            nc.scalar.activation(eu[:], sm[:], AF.Exp, bias=nmx[:, 0:1],
                                 accum_out=zsum[:])
            nc.vector.reciprocal(rz[:], zsum[:])
            nc.vector.tensor_scalar_mul(alphas_n[:], eu[:], rz[:, 0:1])
            if debug:
                nc.sync.dma_start(out=dbg["d_alpha"][:], in_=alphas_n[:])
            # alphasT masked-diagonal expansion [128, 2*S]
            nc.vector.memset(amask[:], 0.0)
            for c in range(2):
                tp = ps_tp.tile([P, NB], F16, tag="tp")
                nc.tensor.transpose(tp[:], alphas_n[:, c * P:(c + 1) * P],
                                    identity_h[0:NB, 0:NB])
                nc.vector.tensor_copy(_diag_out(amask, c * S), tp[:])

        # ---------- A5: ctx for all batches -> [16, 2H] ----------
        with tc.tile_pool(name="encp", bufs=3) as encp, \
             tc.tile_pool(name="ps_cu", bufs=1, space="PSUM") as ps_cu, \
             tc.tile_pool(name="ps_tp2", bufs=2, space="PSUM") as ps_tp2:
            ctxps = ps_cu.tile([NB, TWOH], FP32)
            for b in range(NB):
                enc_t = encp.tile([P, 2 * TWOH], F16, tag="enc")
                nc.sync.dma_start(
                    out=enc_t[:].rearrange("p (c d) -> p c d", c=2),
                    in_=enc_d[b * S:(b + 1) * S, :].rearrange(
                        "(c p) d -> p c d", p=P))
                for sc in range(2):
                    for nh in range(2):
                        nc.tensor.matmul(
                            ctxps[:, nh * H:(nh + 1) * H],
                            amask[:, sc * S + b * NB:sc * S + (b + 1) * NB],
                            enc_t[:, sc * TWOH + nh * H:
                                  sc * TWOH + (nh + 1) * H],
                            start=(b == 0 and sc == 0),
                            stop=(b == 15 and sc == 1))
            nc.vector.tensor_copy(ctx_bm[:], ctxps[:])
            if debug:
                nc.sync.dma_start(out=dbg["d_ctx"][:], in_=ctx_bm[:])
            # ctxT [128, 8*NB]
            for kc in range(8):
                tp = ps_tp2.tile([P, NB], F16, tag="tp")
                nc.tensor.transpose(tp[:], ctx_bm[:, kc * P:(kc + 1) * P],
                                    identity_h[0:NB, 0:NB])
                nc.vector.tensor_copy(ctxT_sb[:, kc * NB:(kc + 1) * NB],
                                      tp[:])

        # ---------- A7/A8: gate consts + out consts (batch-major) ----------
        wcx_sb = bw.tile([P, 8 * FOURH], F16)
        _load_chunked(nc, wcx_sb, wcxT_d, 8, FOURH)
        with tc.tile_pool(name="ps_gc", bufs=1, space="PSUM") as ps_gc, \
             tc.tile_pool(name="ps_oc", bufs=1, space="PSUM") as ps_oc, \
             tc.tile_pool(name="ps_tp3", bufs=2, space="PSUM") as ps_tp3:
            gcps = ps_gc.tile([NB, FOURH], FP32)
            for n in range(4):
                sl = slice(n * H, (n + 1) * H)
                for k in range(4):
                    nc.tensor.matmul(
                        gcps[:, sl], h0T_h[:, k * NB:(k + 1) * NB],
                        whh_sb[:, k * FOURH + n * H:k * FOURH + (n + 1) * H],
                        start=(k == 0), stop=False)
                for kc in range(8):
                    nc.tensor.matmul(
                        gcps[:, sl], ctxT_sb[:, kc * NB:(kc + 1) * NB],
                        wcx_sb[:, kc * FOURH + n * H:kc * FOURH + (n + 1) * H],
                        start=False, stop=False)
                nc.tensor.matmul(gcps[:, sl], ones16[0:1, :],
                                 biasg_sb[0:1, sl], start=False, stop=True)
            nc.vector.tensor_copy(gc_bm[:], gcps[:])
            if debug:
                nc.sync.dma_start(out=dbg["d_gc"][:], in_=gc_bm[:])
            for mg in range(16):
                tp = ps_tp3.tile([P, NB], F16, tag="tp")
                nc.tensor.transpose(tp[:], gc_bm[:, mg * P:(mg + 1) * P],
                                    identity_h[0:NB, 0:NB])
                nc.vector.tensor_copy(gcT_sb[:, mg * NB:(mg + 1) * NB], tp[:])
            ocps = ps_oc.tile([NB, H], FP32)
            for kc in range(8):
                nc.tensor.matmul(ocps[:], ctxT_sb[:, kc * NB:(kc + 1) * NB],
                                 pwc_sb[:, kc * H:(kc + 1) * H],
                                 start=(kc == 0), stop=(kc == 7))
            nc.vector.tensor_copy(oc_sb[:], ocps[:])
            if debug:
                nc.sync.dma_start(out=dbg["d_oc"][:], in_=oc_sb[:])

        # ---------- phase B ----------
        wih8_v = wih8_sb[:].rearrange("p (k n) -> p k n", k=2)
        preW8_v = preW8_sb[:].rearrange("p (k n) -> p k n", k=2)
        with tc.tile_pool(name="embTp", bufs=2) as embTp, \
             tc.tile_pool(name="sgp", bufs=2) as sgp, \
             tc.tile_pool(name="hTp", bufs=2) as hTp, \
             tc.tile_pool(name="outp", bufs=3) as outp, \
             tc.tile_pool(name="ps_tpB", bufs=2, space="PSUM") as ps_tpB, \
             tc.tile_pool(name="ps_g", bufs=1, space="PSUM") as ps_g, \
             tc.tile_pool(name="ps_o", bufs=2, space="PSUM") as ps_o:
            for tt in range(NTT):
                b0 = 2 * tt
                # embT (fp8): [128, 2*512] chunk-major
                embT = embTp.tile([P, TWOH], F8, tag="embT")
                for j in range(4):
                    for e in range(2):
                        tp = ps_tpB.tile([P, 2 * P], F8, tag="tpB")
                        tpa = tp[:]
                        tp2 = bass.AP(tpa.tensor, tpa.offset,
                                      [tpa.ap[0], [2, P]])
                        nc.tensor.transpose(
                            tp2,
                            ge_all[:, (tt * 4 + j) * E + e * P:
                                   (tt * 4 + j) * E + (e + 1) * P],
                            identity_8[:])
                        nc.vector.tensor_copy(
                            embT[:, e * H + j * P:e * H + (j + 1) * P], tp2)
                if debug:
                    nc.sync.dma_start(
                        out=dbg["d_embT"][:, tt * TWOH:(tt + 1) * TWOH],
                        in_=embT[:])
                embT_v = embT[:].rearrange("p (k t) -> p k t", k=2)
                hT_t = hTp.tile([P, 4 * H], F16, tag="hT")
                for hs in range(4):
                    gps = ps_g.tile([P, FOURH], FP32, tag="gps")
                    for g in range(4):
                        mg = g * 4 + hs
                        nc.tensor.matmul(
                            gps[:, g * H:(g + 1) * H],
                            wih8_v[:, :, mg * P:(mg + 1) * P], embT_v,
                            start=True, stop=True, perf_mode=PM.DoubleRow)
                    # bias add (DVE), i/f/o into [P,1536], g into [P,512]
                    sg_ifo = sgp.tile([P, 3 * H], F16, tag="ifo")
                    sg_g = sgp.tile([P, H], F16, tag="g")
                    for gi, g in enumerate((0, 1, 3)):
                        mg = g * 4 + hs
                        nc.vector.tensor_tensor(
                            out=sg_ifo[:, gi * H:(gi + 1) * H],
                            in0=gps[:, g * H:(g + 1) * H],
                            in1=_colpair(gcT_sb, mg * NB + b0, S), op=OP.add)
                    nc.vector.tensor_tensor(
                        out=sg_g[:], in0=gps[:, 2 * H:3 * H],
                        in1=_colpair(gcT_sb, (2 * 4 + hs) * NB + b0, S),
                        op=OP.add)
                    sa_ifo = sgp.tile([P, 3 * H], F16, tag="saifo")
                    nc.scalar.activation(sa_ifo[:], sg_ifo[:], AF.Sigmoid)
                    ta_g = sgp.tile([P, H], F16, tag="tag")
                    nc.scalar.activation(ta_g[:], sg_g[:], AF.Tanh)
                    # c = sig(f)*c0 + sig(i)*tanh(g); h = sig(o)*tanh(c)
                    t1 = sgp.tile([P, H], F16, tag="t1")
                    nc.vector.tensor_tensor(out=t1[:], in0=sa_ifo[:, 0:H],
                                            in1=ta_g[:], op=OP.mult)
                    t2 = sgp.tile([P, H], F16, tag="t2")
                    nc.gpsimd.tensor_tensor(
                        out=t2[:], in0=sa_ifo[:, H:2 * H],
                        in1=_colpair(c0T_sb, hs * NB + b0, S), op=OP.mult)
                    cc = sgp.tile([P, H], F16, tag="cc")
                    nc.vector.tensor_tensor(out=cc[:], in0=t1[:], in1=t2[:],
                                            op=OP.add)
                    tanc = sgp.tile([P, H], F16, tag="tanc")
                    nc.scalar.activation(tanc[:], cc[:], AF.Tanh)
                    nc.vector.tensor_tensor(
                        out=hT_t[:, hs * H:(hs + 1) * H],
                        in0=sa_ifo[:, 2 * H:3 * H], in1=tanc[:], op=OP.mult)
                if debug:
                    nc.sync.dma_start(
                        out=dbg["d_hT"][:, tt * FOURH:(tt + 1) * FOURH],
                        in_=hT_t[:])
                # output projection, straight from PSUM to DRAM
                for tci in range(4):
                    b = b0 + tci // 2
                    po = ps_o.tile([P, H], FP32, tag="po")
                    nc.tensor.matmul(
                        po[:], identity_h[0:NB, b:b + 1].to_broadcast([NB, P]),
                        oc_sb[:], start=True, stop=False)
                    nc.tensor.matmul(
                        po[:], embT_v[:, :, tci * P:(tci + 1) * P], preW8_v,
                        start=False, stop=False, perf_mode=PM.DoubleRow)
                    for k in range(4):
                        nc.tensor.matmul(
                            po[:],
                            hT_t[:, k * H + tci * P:k * H + tci * P + P],
                            preWTh_sb[:, k * H:(k + 1) * H],
                            start=False, stop=(k == 3))
                    o_t = outp.tile([P, H], F16, tag="o")
                    nc.vector.tensor_copy(o_t[:], po[:])
                    nc.sync.dma_start(
                        out=out_d[tt * 512 + tci * P:tt * 512 + (tci + 1) * P,
                                  :],
                        in_=o_t[:])
    return nc


# ---------------------------------------------------------------------------
# host side
# ---------------------------------------------------------------------------

def _chunkmajor(v, chunks, dtype=np.float32):
    return np.ascontiguousarray(v.reshape(chunks, P).T).astype(dtype)


def prep_inputs(inputs, n_cores=N_CORES):
    f32 = lambda x: np.asarray(x, dtype=np.float32)
    f16 = lambda x: np.ascontiguousarray(
        np.asarray(x, dtype=np.float32)).astype(np.float16)
    f8 = lambda x: np.ascontiguousarray(
        np.asarray(x, dtype=np.float32)).astype(ml_dtypes.float8_e4m3fn)
    tgt_seq = np.asarray(inputs["tgt_seq"]).astype(np.int32)
    enc = f32(inputs["encoder_output"])
    eh = f32(inputs["encoder_hidden"])[0]
    ec = f32(inputs["encoder_cell"])[0]
    src_pos = np.asarray(inputs["src_pos"])
    W_ih = f32(inputs["W_ih"])
    pre_W = f32(inputs["pre_W"])
    eW = f32(inputs["energy_W"])[0]

    eWd = np.zeros((P, 4, NB, NB), np.float16)
    for m in range(4):
        blk = eW[m * P:(m + 1) * P].astype(np.float16)
        for b in range(NB):
            eWd[:, m, b, b] = blk
    eWd = np.ascontiguousarray(eWd.reshape(P, 4 * NB * NB))

    shared = dict(
        emb=f8(inputs["emb"]),
        keyWT=f16(f32(inputs["key_W"]).T),
        queryWT=np.ascontiguousarray(f32(inputs["query_W"]).T),
        eWd=eWd,
        wih8=f8(W_ih[:, :E].T),
        whhT=f16(f32(inputs["W_hh"]).T),
        wcxT=f16(W_ih[:, E:].T),
        biasg=f16((f32(inputs["b_ih"]) + f32(inputs["b_hh"]))[None, :]),
        bhWT=np.ascontiguousarray(f32(inputs["bridge_hW"]).T),
        bcWT=np.ascontiguousarray(f32(inputs["bridge_cW"]).T),
        hb=_chunkmajor(f32(inputs["bridge_hb"]), 4),
        cb=_chunkmajor(f32(inputs["bridge_cb"]), 4),
        preW8=f8(pre_W[:, :E].T),
        preWTh=f16(pre_W[:, E:E + H].T),
        preWTc=f16(pre_W[:, E + H:].T),
    )

    in_maps = []
    for i in range(n_cores):
        sl = slice(i * NB, (i + 1) * NB)
        m = src_pos[sl, 0, :].astype(np.float32)
        enc16 = enc[sl].astype(np.float16)          # [NB, S, 2H]
        # k-major encT: [p, (k b s)]
        ekm = np.ascontiguousarray(
            enc16.transpose(2, 0, 1)                 # [2H, NB, S]
            .reshape(8, P, NB, S).transpose(1, 0, 2, 3)
            .reshape(P, 8 * NB * S))
        in_maps.append(dict(
            encTkm=ekm,
            enc=np.ascontiguousarray(enc16.reshape(NB * S, TWOH)),
            ehT=np.ascontiguousarray(eh[sl].T),
            ecT=np.ascontiguousarray(ec[sl].T),
            idx=np.ascontiguousarray(tgt_seq[sl].reshape(-1, P).T),
            mask=np.ascontiguousarray(m),
            maskoff=np.ascontiguousarray(-1e9 * (1.0 - m)),
            **shared,
        ))
    return in_maps, NB


_CACHED = {}


def _get_nc(key=0, debug=False):
    if key not in _CACHED:
        nc = bacc.Bacc("TRN2", target_bir_lowering=False, debug=False)
        build_kernel(nc, debug=debug)
        nc.compile()
        _CACHED[key] = nc
    return _CACHED[key]


def kernel(**inputs):
    in_maps, _ = prep_inputs(inputs, N_CORES)
    nc = _get_nc()
    res = run_bass_kernel_spmd(nc, in_maps, list(range(N_CORES)))
    B = np.asarray(inputs["tgt_seq"]).shape[0]
    out = np.empty((B, T, H), dtype=np.float32)
    for i in range(N_CORES):
        out[i * NB:(i + 1) * NB] = res.results[i]["out"].reshape(NB, T, H)
    return out


# revision 24
# speedup vs baseline: 1.3165x; 1.2565x over previous
"""Trainium2 Bass kernel for the attention-LSTM decoder (restructured).

Computation (all T positions share (h0, c0); see reference):
  h0 = tanh(eh @ bridge_hW.T);  c0 = tanh(ec @ bridge_cW.T)
  energy = tanh(enc @ key_W.T + h0 @ query_W.T);  scores = energy . eW
  alphas = softmax(mask(scores));  ctx = alphas @ enc
  gates = emb[tok] @ W_ih[:,:E].T + [ctx @ W_ih[:,E:].T + h0 @ W_hh.T + b]
  c = sig(f)*c0 + sig(i)*tanh(g);  h = sig(o)*tanh(c)
  out = emb[tok] @ preW[:,:E].T + h @ preW[:,E:E+H].T + ctx @ preW[:,E+H:].T

Sharding: data-parallel over batch B=128 across 8 cores (NB=16 each).

Key structure vs the naive version:
 - keyW projection loops (m,k) outer / batch inner with a k-major host
   layout so each stationary is loaded once and streams N=512 (2 batches).
 - scores land in a single [16,S] PSUM tile via diag-expanded energy_W
   stationaries; softmax runs once on [16,S] rows (exp uses accum_out).
 - ctx for all batches accumulates into one [16,2H] PSUM tile via
   masked-diagonal alphasT stationaries.
 - gate consts / out consts computed batch-major [16,4H]/[16,H] with
   N=512 matmuls, then PE-transposed to the per-partition layouts B needs.
 - phase B: token embeddings gathered in fp8, gates = one fp8 DoubleRow
   matmul per (hs,gate); bias applied on DVE (col-pair broadcast APs);
   activations batched as [128,1536] sigmoid + [128,512] tanh.
 - output projection accumulates oc + emb(fp8 DR) + h(fp16) in PSUM and
   DMAs straight from PSUM to DRAM.
"""

import numpy as np
import ml_dtypes
from contextlib import ExitStack

import concourse.bass as bass
import concourse.mybir as mybir
import concourse.tile as tile
from concourse import bacc
from concourse.bass_utils import run_bass_kernel_spmd
from concourse.masks import make_identity

FP32 = mybir.dt.float32
F16 = mybir.dt.float16
F8 = mybir.dt.float8e4
I32 = mybir.dt.int32
AF = mybir.ActivationFunctionType
OP = mybir.AluOpType
AX = mybir.AxisListType
PM = mybir.MatmulPerfMode

P = 128
H = 512
E = 256
TWOH = 1024
FOURH = 2048
S = 256
T = 256
V = 10000
N_CORES = 8
B_FULL = 128
NB = 16
NTOK = NB * T          # 4096
NTT = NTOK // 512      # 8 token tiles (512 tokens = 2 batches each)


def _load_chunked(nc, dst_tile, src_dram, k_chunks, n):
    """DRAM [k_chunks*128, n] -> SBUF [128, k_chunks*n] (chunk-major)."""
    src = src_dram[:].rearrange("(k p) n -> p k n", p=P)
    dst = dst_tile[:].rearrange("p (k n) -> p k n", k=k_chunks)
    nc.sync.dma_start(out=dst, in_=src)


def _colpair(t, col0, rep):
    """AP reading cols [col0, col0+1] of tile t, each broadcast rep times."""
    ap = t[:]
    return bass.AP(ap.tensor, ap.offset + col0, [ap.ap[0], [1, 2], [0, rep]])


def _diag_out(t, col0):
    """AP writing 16 cols of tile t at col0 + 17*j (block-diagonal)."""
    ap = t[:]
    return bass.AP(ap.tensor, ap.offset + col0, [ap.ap[0], [17, 16]])


def build_kernel(nc, debug=False):
    dt = lambda name, shape, dtype=FP32: nc.dram_tensor(
        name, shape, dtype, kind="ExternalInput")

    encT_d = dt("encTkm", [P, 8 * NB * S], F16)     # [p,(k b s)] k-major
    enc_d = dt("enc", [NB * S, TWOH], F16)          # S-major per batch
    ehT_d = dt("ehT", [TWOH, NB])
    ecT_d = dt("ecT", [TWOH, NB])
    idx_d = dt("idx", [P, NTOK // P], I32)
    mask_d = dt("mask", [NB, S])
    maskoff_d = dt("maskoff", [NB, S])
    emb_d = dt("emb", [V, E], F8)
    keyWT_d = dt("keyWT", [TWOH, H], F16)
    queryWT_d = dt("queryWT", [H, H])
    eWd_d = dt("eWd", [P, 4 * NB * NB], F16)        # diag-expanded energy_W
    wih8_d = dt("wih8", [E, FOURH], F8)
    whhT_d = dt("whhT", [H, FOURH], F16)
    wcxT_d = dt("wcxT", [TWOH, FOURH], F16)
    biasg_d = dt("biasg", [1, FOURH], F16)
    bhWT_d = dt("bhWT", [TWOH, H])
    bcWT_d = dt("bcWT", [TWOH, H])
    hb_d = dt("hb", [P, 4])
    cb_d = dt("cb", [P, 4])
    preW8_d = dt("preW8", [E, H], F8)
    preWTh_d = dt("preWTh", [H, H], F16)
    preWTc_d = dt("preWTc", [TWOH, H], F16)
    out_d = nc.dram_tensor("out", [NTOK, H], F16, kind="ExternalOutput")
    oc_dram = nc.dram_tensor("oc_bounce", [NB, H], F16, kind="Internal")

    dbg = {}
    if debug:
        for name, shape, dty in [
            ("d_energy", [P, 4 * NB * S], F16), ("d_alpha", [NB, S], F16),
            ("d_ctx", [NB, TWOH], F16), ("d_gc", [NB, FOURH], F16),
            ("d_oc", [NB, H], F16), ("d_embT", [P, NTT * TWOH], F8),
            ("d_hT", [P, NTT * FOURH], F16),
        ]:
            dbg[name] = nc.dram_tensor(name, shape, dty, kind="ExternalOutput")

    with ExitStack() as ctx:
        tc = ctx.enter_context(tile.TileContext(nc))

        # ---------- constants ----------
        const = ctx.enter_context(tc.tile_pool(name="const", bufs=1))
        identity_h = const.tile([P, P], F16)
        make_identity(nc, identity_h[:])
        identity_8 = const.tile([P, P], F8)
        nc.vector.tensor_copy(identity_8[:], identity_h[:])
        ones16 = const.tile([1, NB], F16)
        nc.vector.memset(ones16[:], 1.0)

        idx_sb = const.tile([P, NTOK // P], I32)
        nc.sync.dma_start(out=idx_sb[:], in_=idx_d[:])
        mask_sb = const.tile([NB, S], FP32)
        nc.sync.dma_start(out=mask_sb[:], in_=mask_d[:])
        maskoff_sb = const.tile([NB, S], FP32)
        nc.sync.dma_start(out=maskoff_sb[:], in_=maskoff_d[:])
        eWd_sb = const.tile([P, 4 * NB * NB], F16)
        nc.sync.dma_start(out=eWd_sb[:], in_=eWd_d[:])
        biasg_sb = const.tile([1, FOURH], F16)
        nc.sync.dma_start(out=biasg_sb[:], in_=biasg_d[:])
        hb_sb = const.tile([P, 4], FP32)
        nc.sync.dma_start(out=hb_sb[:], in_=hb_d[:])
        cb_sb = const.tile([P, 4], FP32)
        nc.sync.dma_start(out=cb_sb[:], in_=cb_d[:])
        ehT_sb = const.tile([P, 8 * NB], FP32)
        _load_chunked(nc, ehT_sb, ehT_d, 8, NB)
        ecT_sb = const.tile([P, 8 * NB], FP32)
        _load_chunked(nc, ecT_sb, ecT_d, 8, NB)

        # ---------- token embedding gather (fp8), issued up front ----------
        gep = ctx.enter_context(tc.tile_pool(name="gep", bufs=1))
        ge_all = gep.tile([P, NTOK // P * E], F8)
        for j in range(NTOK // P):
            nc.gpsimd.indirect_dma_start(
                out=ge_all[:, j * E:(j + 1) * E], out_offset=None,
                in_=emb_d[:],
                in_offset=bass.IndirectOffsetOnAxis(
                    ap=idx_sb[:, j:j + 1], axis=0))

        # ---------- state ----------
        state = ctx.enter_context(tc.tile_pool(name="state", bufs=1))
        h0T_sb = state.tile([P, 4 * NB], FP32)
        c0T_sb = state.tile([P, 4 * NB], FP32)
        qprojT_sb = state.tile([P, 4 * NB], FP32)
        h0T_h = state.tile([P, 4 * NB], F16)
        alphas_n = state.tile([NB, S], F16)
        amask = state.tile([P, 2 * S], F16)
        ctx_bm = state.tile([NB, TWOH], F16)
        ctxT_sb = state.tile([P, 8 * NB], F16)
        gc_bm = state.tile([NB, FOURH], F16)
        gcT_sb = state.tile([P, 16 * NB], FP32)
        oc_sb = state.tile([NB, H], F16)
        zsum = state.tile([NB, 1], FP32)
        rz = state.tile([NB, 1], FP32)
        nmx = state.tile([NB, 1], FP32)

        # ---------- setup: bridge h0/c0, qproj (fp32) ----------
        with tc.tile_pool(name="setup_w", bufs=1) as swp, \
             tc.tile_pool(name="setup_ps", bufs=2, space="PSUM") as sps:
            bhWT_sb = swp.tile([P, 8 * H], FP32, tag="bridge")
            _load_chunked(nc, bhWT_sb, bhWT_d, 8, H)
            for m in range(4):
                ps = sps.tile([P, NB], FP32, tag="ps")
                for k in range(8):
                    nc.tensor.matmul(
                        ps[:], bhWT_sb[:, k * H + m * P:k * H + m * P + P],
                        ehT_sb[:, k * NB:(k + 1) * NB],
                        start=(k == 0), stop=(k == 7))
                nc.scalar.activation(h0T_sb[:, m * NB:(m + 1) * NB], ps[:],
                                     AF.Tanh, bias=hb_sb[:, m:m + 1])
            bcWT_sb = swp.tile([P, 8 * H], FP32, tag="bridge")
            _load_chunked(nc, bcWT_sb, bcWT_d, 8, H)
            for m in range(4):
                ps = sps.tile([P, NB], FP32, tag="ps")
                for k in range(8):
                    nc.tensor.matmul(
                        ps[:], bcWT_sb[:, k * H + m * P:k * H + m * P + P],
                        ecT_sb[:, k * NB:(k + 1) * NB],
                        start=(k == 0), stop=(k == 7))
                nc.scalar.activation(c0T_sb[:, m * NB:(m + 1) * NB], ps[:],
                                     AF.Tanh, bias=cb_sb[:, m:m + 1])
            qWT_sb = swp.tile([P, 4 * H], FP32, tag="bridge")
            _load_chunked(nc, qWT_sb, queryWT_d, 4, H)
            for m in range(4):
                ps = sps.tile([P, NB], FP32, tag="ps")
                for k in range(4):
                    nc.tensor.matmul(
                        ps[:], qWT_sb[:, k * H + m * P:k * H + m * P + P],
                        h0T_sb[:, k * NB:(k + 1) * NB],
                        start=(k == 0), stop=(k == 3))
                nc.vector.tensor_copy(qprojT_sb[:, m * NB:(m + 1) * NB], ps[:])
            nc.vector.tensor_copy(h0T_h[:], h0T_sb[:])

        # ---------- A1: keyW projection -> energy (fp16) ----------
        ea = ctx.enter_context(tc.tile_pool(name="energy", bufs=1))
        energy_all = ea.tile([P, 4 * NB * S], F16)
        with tc.tile_pool(name="kw", bufs=1) as kwp, \
             tc.tile_pool(name="ps_pk", bufs=1, space="PSUM") as ps_pk:
            keyWT_sb = kwp.tile([P, 8 * H], F16)
            _load_chunked(nc, keyWT_sb, keyWT_d, 8, H)
            encT_sb = kwp.tile([P, 8 * NB * S], F16)
            for k in range(8):
                sl = slice(k * NB * S, (k + 1) * NB * S)
                nc.sync.dma_start(out=encT_sb[:, sl], in_=encT_d[:, sl])
            pk_tiles = [ps_pk.tile([P, 2 * S], FP32, tag=f"pk{i}",
                                   name=f"pk{i}") for i in range(8)]
            for m in range(4):
                for k in range(8):
                    stat = keyWT_sb[:, k * H + m * P:k * H + m * P + P]
                    for bp in range(8):
                        nc.tensor.matmul(
                            pk_tiles[bp][:], stat,
                            encT_sb[:, (k * NB + 2 * bp) * S:
                                    (k * NB + 2 * bp + 2) * S],
                            start=(k == 0), stop=(k == 7))
                for b in range(NB):
                    nc.scalar.activation(
                        energy_all[:, (m * NB + b) * S:(m * NB + b + 1) * S],
                        pk_tiles[b // 2][:, (b % 2) * S:(b % 2 + 1) * S],
                        AF.Tanh, bias=qprojT_sb[:, m * NB + b:m * NB + b + 1])
        if debug:
            nc.sync.dma_start(out=dbg["d_energy"][:], in_=energy_all[:])

        # phase-B weights: pool opened now that the A1 tiles are freed
        bw = ctx.enter_context(tc.tile_pool(name="bw", bufs=1))
        wih8_sb = bw.tile([P, 2 * FOURH], F8)
        _load_chunked(nc, wih8_sb, wih8_d, 2, FOURH)
        whh_sb = bw.tile([P, 4 * FOURH], F16)
        _load_chunked(nc, whh_sb, whhT_d, 4, FOURH)
        preW8_sb = bw.tile([P, 2 * H], F8)
        _load_chunked(nc, preW8_sb, preW8_d, 2, H)
        preWTh_sb = bw.tile([P, 4 * H], F16)
        _load_chunked(nc, preWTh_sb, preWTh_d, 4, H)
        pwc_sb = bw.tile([P, 8 * H], F16)
        _load_chunked(nc, pwc_sb, preWTc_d, 8, H)
        ocb_sb = bw.tile([P, NB * H], F16)

        # ---------- A2+A3: scores [16,S] + softmax ----------
        with tc.tile_pool(name="smx", bufs=1) as smx, \
             tc.tile_pool(name="ps_sc", bufs=1, space="PSUM") as ps_sc, \
             tc.tile_pool(name="ps_tp", bufs=2, space="PSUM") as ps_tp:
            scps = ps_sc.tile([NB, S], FP32)
            for m in range(4):
                for b in range(NB):
                    nc.tensor.matmul(
                        scps[:],
                        eWd_sb[:, (m * NB + b) * NB:(m * NB + b + 1) * NB],
                        energy_all[:, (m * NB + b) * S:(m * NB + b + 1) * S],
                        start=(m == 0 and b == 0), stop=(m == 3 and b == 15))
            sm = smx.tile([NB, S], FP32)
            nc.vector.tensor_tensor(out=sm[:], in0=scps[:], in1=mask_sb[:],
                                    op=OP.mult)
            nc.vector.tensor_tensor(out=sm[:], in0=sm[:], in1=maskoff_sb[:],
                                    op=OP.add)
            nc.vector.tensor_reduce(nmx[:], sm[:], AX.X, OP.max, negate=True)
            eu = smx.tile([NB, S], F16)
            nc.scalar.activation(eu[:], sm[:], AF.Exp, bias=nmx[:, 0:1],
                                 accum_out=zsum[:])
            nc.vector.reciprocal(rz[:], zsum[:])
            nc.vector.tensor_scalar_mul(alphas_n[:], eu[:], rz[:, 0:1])
            if debug:
                nc.sync.dma_start(out=dbg["d_alpha"][:], in_=alphas_n[:])
            # alphasT masked-diagonal expansion [128, 2*S]
            nc.vector.memset(amask[:], 0.0)
            tpa2 = ps_tp.tile([P, 2 * NB], F16, tag="tp")
            for c in range(2):
                nc.tensor.transpose(tpa2[:, c * NB:(c + 1) * NB],
                                    alphas_n[:, c * P:(c + 1) * P],
                                    identity_h[0:NB, 0:NB])
            for c in range(2):
                nc.vector.tensor_copy(_diag_out(amask, c * S),
                                      tpa2[:, c * NB:(c + 1) * NB])

        # ---------- A5: ctx for all batches -> [16, 2H] ----------
        with tc.tile_pool(name="encp", bufs=3) as encp, \
             tc.tile_pool(name="ps_cu", bufs=1, space="PSUM") as ps_cu, \
             tc.tile_pool(name="ps_tp2", bufs=2, space="PSUM") as ps_tp2:
            ctxps = ps_cu.tile([NB, TWOH], FP32)
            for b in range(NB):
                enc_t = encp.tile([P, 2 * TWOH], F16, tag="enc")
                nc.sync.dma_start(
                    out=enc_t[:].rearrange("p (c d) -> p c d", c=2),
                    in_=enc_d[b * S:(b + 1) * S, :].rearrange(
                        "(c p) d -> p c d", p=P))
                for sc in range(2):
                    for nh in range(2):
                        nc.tensor.matmul(
                            ctxps[:, nh * H:(nh + 1) * H],
                            amask[:, sc * S + b * NB:sc * S + (b + 1) * NB],
                            enc_t[:, sc * TWOH + nh * H:
                                  sc * TWOH + (nh + 1) * H],
                            start=(b == 0 and sc == 0),
                            stop=(b == 15 and sc == 1))
            nc.vector.tensor_copy(ctx_bm[:], ctxps[:])
            if debug:
                nc.sync.dma_start(out=dbg["d_ctx"][:], in_=ctx_bm[:])
            # ctxT [128, 8*NB] via one batched transpose+copy
            tpc = ps_tp2.tile([P, 8 * NB], F16, tag="tp")
            for kc in range(8):
                nc.tensor.transpose(tpc[:, kc * NB:(kc + 1) * NB],
                                    ctx_bm[:, kc * P:(kc + 1) * P],
                                    identity_h[0:NB, 0:NB])
            nc.vector.tensor_copy(ctxT_sb[:], tpc[:])

        # ---------- A7/A8: gate consts + out consts (batch-major) ----------
        wcx_sb = bw.tile([P, 8 * FOURH], F16)
        _load_chunked(nc, wcx_sb, wcxT_d, 8, FOURH)
        with tc.tile_pool(name="ps_gc", bufs=1, space="PSUM") as ps_gc, \
             tc.tile_pool(name="ps_oc", bufs=1, space="PSUM") as ps_oc, \
             tc.tile_pool(name="ps_tp3", bufs=2, space="PSUM") as ps_tp3:
            gcps = ps_gc.tile([NB, FOURH], FP32)
            for n in range(4):
                sl = slice(n * H, (n + 1) * H)
                for k in range(4):
                    nc.tensor.matmul(
                        gcps[:, sl], h0T_h[:, k * NB:(k + 1) * NB],
                        whh_sb[:, k * FOURH + n * H:k * FOURH + (n + 1) * H],
                        start=(k == 0), stop=False)
                for kc in range(8):
                    nc.tensor.matmul(
                        gcps[:, sl], ctxT_sb[:, kc * NB:(kc + 1) * NB],
                        wcx_sb[:, kc * FOURH + n * H:kc * FOURH + (n + 1) * H],
                        start=False, stop=False)
                nc.tensor.matmul(gcps[:, sl], ones16[0:1, :],
                                 biasg_sb[0:1, sl], start=False, stop=True)
            nc.vector.tensor_copy(gc_bm[:], gcps[:])
            if debug:
                nc.sync.dma_start(out=dbg["d_gc"][:], in_=gc_bm[:])
            for half in range(2):
                tp = ps_tp3.tile([P, 8 * NB], F16, tag="tp", name=f"tpg{half}")
                for j in range(8):
                    mg = half * 8 + j
                    nc.tensor.transpose(tp[:, j * NB:(j + 1) * NB],
                                        gc_bm[:, mg * P:(mg + 1) * P],
                                        identity_h[0:NB, 0:NB])
                nc.vector.tensor_copy(
                    gcT_sb[:, half * 8 * NB:(half + 1) * 8 * NB], tp[:])
            ocps = ps_oc.tile([NB, H], FP32)
            for kc in range(8):
                nc.tensor.matmul(ocps[:], ctxT_sb[:, kc * NB:(kc + 1) * NB],
                                 pwc_sb[:, kc * H:(kc + 1) * H],
                                 start=(kc == 0), stop=(kc == 7))
            nc.vector.tensor_copy(oc_sb[:], ocps[:])
            if debug:
                nc.sync.dma_start(out=dbg["d_oc"][:], in_=oc_sb[:])
            # broadcast oc rows to all 128 partitions via DRAM bounce
            nc.sync.dma_start(out=oc_dram[:], in_=oc_sb[:])
            nc.sync.dma_start(
                out=ocb_sb[:].rearrange("p (b n) -> p b n", b=NB),
                in_=bass.AP(oc_dram[:].tensor, 0,
                            [[0, P], [H, NB], [1, H]]))

        # ---------- phase B ----------
        # Per ttile (512 tokens = batches b0,b0+1):
        #   gates via fp8 DoubleRow matmuls into [P,1024] psum pairs
        #   (pair A = i,g; pair B = f,o); sigmoid/tanh on Scalar with
        #   per-(gate,batch) bias column from gcT; LSTM combine skewed one
        #   hs behind so no engine stalls on the serial chain.
        wih8_v = wih8_sb[:].rearrange("p (k n) -> p k n", k=2)
        preW8_v = preW8_sb[:].rearrange("p (k n) -> p k n", k=2)
        with tc.tile_pool(name="embTp", bufs=2) as embTp, \
             tc.tile_pool(name="sgp", bufs=3) as sgp, \
             tc.tile_pool(name="hTp", bufs=2) as hTp, \
             tc.tile_pool(name="outp", bufs=3) as outp, \
             tc.tile_pool(name="ps_tpB", bufs=2, space="PSUM") as ps_tpB, \
             tc.tile_pool(name="ps_g", bufs=1, space="PSUM") as ps_g, \
             tc.tile_pool(name="ps_o", bufs=2, space="PSUM") as ps_o:

            def gate_mm(gps, half, hs, embT_v):
                # half 0 -> gates (i, g) in slices 0/1; half 1 -> (f, o)
                gs = (0, 2) if half == 0 else (1, 3)
                for sl, g in enumerate(gs):
                    mg = g * 4 + hs
                    nc.tensor.matmul(
                        gps[:, sl * H:(sl + 1) * H],
                        wih8_v[:, :, mg * P:(mg + 1) * P], embT_v,
                        start=True, stop=True, perf_mode=PM.DoubleRow)

            def gate_act(gps, half, hs, b0, out_t):
                gs = (0, 2) if half == 0 else (1, 3)
                for sl, g in enumerate(gs):
                    mg = g * 4 + hs
                    fn = AF.Tanh if g == 2 else AF.Sigmoid
                    for x in range(2):
                        cs = slice(sl * H + x * S, sl * H + (x + 1) * S)
                        nc.scalar.activation(
                            out_t[:, cs], gps[:, cs], fn,
                            bias=gcT_sb[:, mg * NB + b0 + x:
                                        mg * NB + b0 + x + 1])

            for tt in range(NTT):
                b0 = 2 * tt
                # embT (fp8): [128, 2*512] chunk-major
                embT = embTp.tile([P, TWOH], F8, tag="embT")
                for j in range(4):
                    for e in range(2):
                        tp = ps_tpB.tile([P, 2 * P], F8, tag="tpB")
                        tpa = tp[:]
                        tp2 = bass.AP(tpa.tensor, tpa.offset,
                                      [tpa.ap[0], [2, P]])
                        nc.tensor.transpose(
                            tp2,
                            ge_all[:, (tt * 4 + j) * E + e * P:
                                   (tt * 4 + j) * E + (e + 1) * P],
                            identity_8[:])
                        nc.vector.tensor_copy(
                            embT[:, e * H + j * P:e * H + (j + 1) * P], tp2)
                if debug:
                    nc.sync.dma_start(
                        out=dbg["d_embT"][:, tt * TWOH:(tt + 1) * TWOH],
                        in_=embT[:])
                embT_v = embT[:].rearrange("p (k t) -> p k t", k=2)
                hT_t = hTp.tile([P, 4 * H], F16, tag="hT")

                stage = {}   # hs -> (sIG, sFO, t1, t2, cc)
                def combine_tail(hs):
                    sIG, sFO, t1, t2, cc = stage.pop(hs)
                    tanc = sgp.tile([P, H], F16, tag="tanc", name="tanc")
                    nc.scalar.activation(tanc[:], cc[:], AF.Tanh)
                    nc.vector.tensor_tensor(
                        out=hT_t[:, hs * H:(hs + 1) * H],
                        in0=sFO[:, H:2 * H], in1=tanc[:], op=OP.mult)

                for hs in range(4):
                    gpsA = ps_g.tile([P, TWOH], FP32, tag="gA", name="gA")
                    gate_mm(gpsA, 0, hs, embT_v)
                    gpsB = ps_g.tile([P, TWOH], FP32, tag="gB", name="gB")
                    gate_mm(gpsB, 1, hs, embT_v)
                    sIG = sgp.tile([P, TWOH], F16, tag="sIG", name="sIG")
                    gate_act(gpsA, 0, hs, b0, sIG)
                    sFO = sgp.tile([P, TWOH], F16, tag="sFO", name="sFO")
                    gate_act(gpsB, 1, hs, b0, sFO)
                    t1 = sgp.tile([P, H], F16, tag="t1", name="t1")
                    nc.vector.tensor_tensor(out=t1[:], in0=sIG[:, 0:H],
                                            in1=sIG[:, H:TWOH], op=OP.mult)
                    t2 = sgp.tile([P, H], F16, tag="t2", name="t2")
                    nc.gpsimd.tensor_tensor(
                        out=t2[:], in0=sFO[:, 0:H],
                        in1=_colpair(c0T_sb, hs * NB + b0, S), op=OP.mult)
                    cc = sgp.tile([P, H], F16, tag="cc", name="cc")
                    nc.vector.tensor_tensor(out=cc[:], in0=t1[:], in1=t2[:],
                                            op=OP.add)
                    stage[hs] = (sIG, sFO, t1, t2, cc)
                    if hs > 0:
                        combine_tail(hs - 1)
                combine_tail(3)
                if debug:
                    nc.sync.dma_start(
                        out=dbg["d_hT"][:, tt * FOURH:(tt + 1) * FOURH],
                        in_=hT_t[:])
                # output projection: emb(fp8 DR) + h(fp16); oc added on DVE
                for tci in range(4):
                    b = b0 + tci // 2
                    po = ps_o.tile([P, H], FP32, tag="po")
                    nc.tensor.matmul(
                        po[:], embT_v[:, :, tci * P:(tci + 1) * P], preW8_v,
                        start=True, stop=False, perf_mode=PM.DoubleRow)
                    for k in range(4):
                        nc.tensor.matmul(
                            po[:],
                            hT_t[:, k * H + tci * P:k * H + tci * P + P],
                            preWTh_sb[:, k * H:(k + 1) * H],
                            start=False, stop=(k == 3))
                    o_t = outp.tile([P, H], F16, tag="o")
                    nc.vector.tensor_tensor(
                        out=o_t[:], in0=po[:],
                        in1=ocb_sb[:, b * H:(b + 1) * H], op=OP.add)
                    nc.sync.dma_start(
                        out=out_d[tt * 512 + tci * P:tt * 512 + (tci + 1) * P,
                                  :],
                        in_=o_t[:])
    return nc


# ---------------------------------------------------------------------------
# host side
# ---------------------------------------------------------------------------

def _chunkmajor(v, chunks, dtype=np.float32):
    return np.ascontiguousarray(v.reshape(chunks, P).T).astype(dtype)


def prep_inputs(inputs, n_cores=N_CORES):
    f32 = lambda x: np.asarray(x, dtype=np.float32)
    f16 = lambda x: np.ascontiguousarray(
        np.asarray(x, dtype=np.float32)).astype(np.float16)
    f8 = lambda x: np.ascontiguousarray(
        np.asarray(x, dtype=np.float32)).astype(ml_dtypes.float8_e4m3fn)
    tgt_seq = np.asarray(inputs["tgt_seq"]).astype(np.int32)
    enc = f32(inputs["encoder_output"])
    eh = f32(inputs["encoder_hidden"])[0]
    ec = f32(inputs["encoder_cell"])[0]
    src_pos = np.asarray(inputs["src_pos"])
    W_ih = f32(inputs["W_ih"])
    pre_W = f32(inputs["pre_W"])
    eW = f32(inputs["energy_W"])[0]

    eWd = np.zeros((P, 4, NB, NB), np.float16)
    for m in range(4):
        blk = eW[m * P:(m + 1) * P].astype(np.float16)
        for b in range(NB):
            eWd[:, m, b, b] = blk
    eWd = np.ascontiguousarray(eWd.reshape(P, 4 * NB * NB))

    shared = dict(
        emb=f8(inputs["emb"]),
        keyWT=f16(f32(inputs["key_W"]).T),
        queryWT=np.ascontiguousarray(f32(inputs["query_W"]).T),
        eWd=eWd,
        wih8=f8(W_ih[:, :E].T),
        whhT=f16(f32(inputs["W_hh"]).T),
        wcxT=f16(W_ih[:, E:].T),
        biasg=f16((f32(inputs["b_ih"]) + f32(inputs["b_hh"]))[None, :]),
        bhWT=np.ascontiguousarray(f32(inputs["bridge_hW"]).T),
        bcWT=np.ascontiguousarray(f32(inputs["bridge_cW"]).T),
        hb=_chunkmajor(f32(inputs["bridge_hb"]), 4),
        cb=_chunkmajor(f32(inputs["bridge_cb"]), 4),
        preW8=f8(pre_W[:, :E].T),
        preWTh=f16(pre_W[:, E:E + H].T),
        preWTc=f16(pre_W[:, E + H:].T),
    )

    in_maps = []
    for i in range(n_cores):
        sl = slice(i * NB, (i + 1) * NB)
        m = src_pos[sl, 0, :].astype(np.float32)
        enc16 = enc[sl].astype(np.float16)          # [NB, S, 2H]
        # k-major encT: [p, (k b s)]
        ekm = np.ascontiguousarray(
            enc16.transpose(2, 0, 1)                 # [2H, NB, S]
            .reshape(8, P, NB, S).transpose(1, 0, 2, 3)
            .reshape(P, 8 * NB * S))
        in_maps.append(dict(
            encTkm=ekm,
            enc=np.ascontiguousarray(enc16.reshape(NB * S, TWOH)),
            ehT=np.ascontiguousarray(eh[sl].T),
            ecT=np.ascontiguousarray(ec[sl].T),
            idx=np.ascontiguousarray(tgt_seq[sl].reshape(-1, P).T),
            mask=np.ascontiguousarray(m),
            maskoff=np.ascontiguousarray(-1e9 * (1.0 - m)),
            **shared,
        ))
    return in_maps, NB


_CACHED = {}


def _get_nc(key=0, debug=False):
    if key not in _CACHED:
        nc = bacc.Bacc("TRN2", target_bir_lowering=False, debug=False)
        build_kernel(nc, debug=debug)
        nc.compile()
        _CACHED[key] = nc
    return _CACHED[key]


def kernel(**inputs):
    in_maps, _ = prep_inputs(inputs, N_CORES)
    nc = _get_nc()
    res = run_bass_kernel_spmd(nc, in_maps, list(range(N_CORES)))
    B = np.asarray(inputs["tgt_seq"]).shape[0]
    out = np.empty((B, T, H), dtype=np.float32)
    for i in range(N_CORES):
        out[i * NB:(i + 1) * NB] = res.results[i]["out"].reshape(NB, T, H)
    return out


# revision 34
# speedup vs baseline: 1.3305x; 1.0106x over previous
"""Trainium2 Bass kernel for the attention-LSTM decoder (restructured).

Computation (all T positions share (h0, c0); see reference):
  h0 = tanh(eh @ bridge_hW.T);  c0 = tanh(ec @ bridge_cW.T)
  energy = tanh(enc @ key_W.T + h0 @ query_W.T);  scores = energy . eW
  alphas = softmax(mask(scores));  ctx = alphas @ enc
  gates = emb[tok] @ W_ih[:,:E].T + [ctx @ W_ih[:,E:].T + h0 @ W_hh.T + b]
  c = sig(f)*c0 + sig(i)*tanh(g);  h = sig(o)*tanh(c)
  out = emb[tok] @ preW[:,:E].T + h @ preW[:,E:E+H].T + ctx @ preW[:,E+H:].T

Sharding: data-parallel over batch B=128 across 8 cores (NB=16 each).

Key structure vs the naive version:
 - keyW projection loops (m,k) outer / batch inner with a k-major host
   layout so each stationary is loaded once and streams N=512 (2 batches).
 - scores land in a single [16,S] PSUM tile via diag-expanded energy_W
   stationaries; softmax runs once on [16,S] rows (exp uses accum_out).
 - ctx for all batches accumulates into one [16,2H] PSUM tile via
   masked-diagonal alphasT stationaries.
 - gate consts / out consts computed batch-major [16,4H]/[16,H] with
   N=512 matmuls, then PE-transposed to the per-partition layouts B needs.
 - phase B: token embeddings gathered in fp8, gates = one fp8 DoubleRow
   matmul per (hs,gate); bias applied on DVE (col-pair broadcast APs);
   activations batched as [128,1536] sigmoid + [128,512] tanh.
 - output projection accumulates oc + emb(fp8 DR) + h(fp16) in PSUM and
   DMAs straight from PSUM to DRAM.
"""

import numpy as np
import ml_dtypes
from contextlib import ExitStack

import concourse.bass as bass
import concourse.mybir as mybir
import concourse.tile as tile
from concourse import bacc
from concourse.bass_utils import run_bass_kernel_spmd
from concourse.masks import make_identity

FP32 = mybir.dt.float32
F16 = mybir.dt.float16
F8 = mybir.dt.float8e4
I32 = mybir.dt.int32
AF = mybir.ActivationFunctionType
OP = mybir.AluOpType
AX = mybir.AxisListType
PM = mybir.MatmulPerfMode

P = 128
H = 512
E = 256
TWOH = 1024
FOURH = 2048
S = 256
T = 256
V = 10000
N_CORES = 8
B_FULL = 128
NB = 16
NTOK = NB * T          # 4096
NTT = NTOK // 512      # 8 token tiles (512 tokens = 2 batches each)


def _load_chunked(nc, dst_tile, src_dram, k_chunks, n):
    """DRAM [k_chunks*128, n] -> SBUF [128, k_chunks*n] (chunk-major)."""
    src = src_dram[:].rearrange("(k p) n -> p k n", p=P)
    dst = dst_tile[:].rearrange("p (k n) -> p k n", k=k_chunks)
    nc.sync.dma_start(out=dst, in_=src)


def _colpair(t, col0, rep):
    """AP reading cols [col0, col0+1] of tile t, each broadcast rep times."""
    ap = t[:]
    return bass.AP(ap.tensor, ap.offset + col0, [ap.ap[0], [1, 2], [0, rep]])


def _diag_out(t, col0):
    """AP writing 16 cols of tile t at col0 + 17*j (block-diagonal)."""
    ap = t[:]
    return bass.AP(ap.tensor, ap.offset + col0, [ap.ap[0], [17, 16]])


def build_kernel(nc, debug=False):
    dt = lambda name, shape, dtype=FP32: nc.dram_tensor(
        name, shape, dtype, kind="ExternalInput")

    encT_d = dt("encTkm", [P, 8 * NB * S], F16)     # [p,(k b s)] k-major
    enc_d = dt("enc", [NB * S, TWOH], F16)          # S-major per batch
    # bridge path in fp16 (fp32 matmuls are 4 cycles/row on PE)
    ehT_d = dt("ehT", [TWOH, NB], F16)
    ecT_d = dt("ecT", [TWOH, NB], F16)
    idx_d = dt("idx", [P, NTOK // P], I32)
    mask_d = dt("mask", [NB, S])
    maskoff_d = dt("maskoff", [NB, S])
    emb_d = dt("emb", [V, E], F8)
    keyWT_d = dt("keyWT", [TWOH, H], F16)
    queryWT_d = dt("queryWT", [H, H], F16)
    eWd_d = dt("eWd", [P, 4 * NB * NB], F16)        # diag-expanded energy_W
    wih8_d = dt("wih8", [E, FOURH], F8)
    whhT_d = dt("whhT", [H, FOURH], F16)
    wcxT_d = dt("wcxT", [TWOH, FOURH], F16)
    biasg_d = dt("biasg", [1, FOURH], F16)
    bhWT_d = dt("bhWT", [TWOH, H], F16)
    bcWT_d = dt("bcWT", [TWOH, H], F16)
    hb_d = dt("hb", [P, 4])
    cb_d = dt("cb", [P, 4])
    preW8_d = dt("preW8", [E, H], F8)
    preWTh_d = dt("preWTh", [H, H], F16)
    preWTc_d = dt("preWTc", [TWOH, H], F16)
    out_d = nc.dram_tensor("out", [NTOK, H], F16, kind="ExternalOutput")
    oc_dram = nc.dram_tensor("oc_bounce", [NB, H], F16, kind="Internal")

    dbg = {}
    if debug:
        for name, shape, dty in [
            ("d_energy", [P, 4 * NB * S], F16), ("d_alpha", [NB, S], F16),
            ("d_ctx", [NB, TWOH], F16), ("d_gc", [NB, FOURH], F16),
            ("d_oc", [NB, H], F16), ("d_embT", [P, NTT * TWOH], F8),
            ("d_hT", [P, NTT * FOURH], F16),
        ]:
            dbg[name] = nc.dram_tensor(name, shape, dty, kind="ExternalOutput")

    with ExitStack() as ctx:
        tc = ctx.enter_context(tile.TileContext(nc))

        # ---------- constants ----------
        const = ctx.enter_context(tc.tile_pool(name="const", bufs=1))
        identity_h = const.tile([P, P], F16)
        make_identity(nc, identity_h[:])
        identity_8 = const.tile([P, P], F8)
        nc.vector.tensor_copy(identity_8[:], identity_h[:])
        ones16 = const.tile([1, NB], F16)
        nc.vector.memset(ones16[:], 1.0)

        idx_sb = const.tile([P, NTOK // P], I32)
        nc.sync.dma_start(out=idx_sb[:], in_=idx_d[:])
        mask_sb = const.tile([NB, S], FP32)
        nc.sync.dma_start(out=mask_sb[:], in_=mask_d[:])
        maskoff_sb = const.tile([NB, S], FP32)
        nc.sync.dma_start(out=maskoff_sb[:], in_=maskoff_d[:])
        eWd_sb = const.tile([P, 4 * NB * NB], F16)
        nc.sync.dma_start(out=eWd_sb[:], in_=eWd_d[:])
        biasg_sb = const.tile([1, FOURH], F16)
        nc.sync.dma_start(out=biasg_sb[:], in_=biasg_d[:])
        hb_sb = const.tile([P, 4], FP32)
        nc.sync.dma_start(out=hb_sb[:], in_=hb_d[:])
        cb_sb = const.tile([P, 4], FP32)
        nc.sync.dma_start(out=cb_sb[:], in_=cb_d[:])
        ehT_sb = const.tile([P, 8 * NB], F16)
        _load_chunked(nc, ehT_sb, ehT_d, 8, NB)
        ecT_sb = const.tile([P, 8 * NB], F16)
        _load_chunked(nc, ecT_sb, ecT_d, 8, NB)

        # ---------- token embedding gather (fp8), issued up front ----------
        gep = ctx.enter_context(tc.tile_pool(name="gep", bufs=1))
        ge_all = gep.tile([P, NTOK // P * E], F8)
        for j in range(NTOK // P):
            nc.gpsimd.indirect_dma_start(
                out=ge_all[:, j * E:(j + 1) * E], out_offset=None,
                in_=emb_d[:],
                in_offset=bass.IndirectOffsetOnAxis(
                    ap=idx_sb[:, j:j + 1], axis=0))

        # ---------- state ----------
        state = ctx.enter_context(tc.tile_pool(name="state", bufs=1))
        h0T_sb = state.tile([P, 4 * NB], FP32)
        c0T_sb = state.tile([P, 4 * NB], FP32)
        qprojT_sb = state.tile([P, 4 * NB], FP32)
        h0T_h = state.tile([P, 4 * NB], F16)
        c0T_h = state.tile([P, 4 * NB], F16)
        alphas_n = state.tile([NB, S], F16)
        amask = state.tile([P, 2 * S], F16)
        ctx_bm = state.tile([NB, TWOH], F16)
        ctxT_sb = state.tile([P, 8 * NB], F16)
        gc_bm = state.tile([NB, FOURH], F16)
        gcT_sb = state.tile([P, 16 * NB], FP32)
        oc_sb = state.tile([NB, H], F16)
        zsum = state.tile([NB, 1], FP32)
        rz = state.tile([NB, 1], FP32)
        nmx = state.tile([NB, 1], FP32)

        # ---------- setup: bridge h0/c0, qproj (fp32) ----------
        with tc.tile_pool(name="setup_w", bufs=1) as swp, \
             tc.tile_pool(name="setup_ps", bufs=2, space="PSUM") as sps:
            bhWT_sb = swp.tile([P, 8 * H], F16, tag="bridge")
            _load_chunked(nc, bhWT_sb, bhWT_d, 8, H)
            for m in range(4):
                ps = sps.tile([P, NB], FP32, tag="ps")
                for k in range(8):
                    nc.tensor.matmul(
                        ps[:], bhWT_sb[:, k * H + m * P:k * H + m * P + P],
                        ehT_sb[:, k * NB:(k + 1) * NB],
                        start=(k == 0), stop=(k == 7))
                nc.scalar.activation(h0T_sb[:, m * NB:(m + 1) * NB], ps[:],
                                     AF.Tanh, bias=hb_sb[:, m:m + 1])
            nc.vector.tensor_copy(h0T_h[:], h0T_sb[:])
            bcWT_sb = swp.tile([P, 8 * H], F16, tag="bridge")
            _load_chunked(nc, bcWT_sb, bcWT_d, 8, H)
            for m in range(4):
                ps = sps.tile([P, NB], FP32, tag="ps")
                for k in range(8):
                    nc.tensor.matmul(
                        ps[:], bcWT_sb[:, k * H + m * P:k * H + m * P + P],
                        ecT_sb[:, k * NB:(k + 1) * NB],
                        start=(k == 0), stop=(k == 7))
                nc.scalar.activation(c0T_sb[:, m * NB:(m + 1) * NB], ps[:],
                                     AF.Tanh, bias=cb_sb[:, m:m + 1])
            qWT_sb = swp.tile([P, 4 * H], F16, tag="bridge")
            _load_chunked(nc, qWT_sb, queryWT_d, 4, H)
            for m in range(4):
                ps = sps.tile([P, NB], FP32, tag="ps")
                for k in range(4):
                    nc.tensor.matmul(
                        ps[:], qWT_sb[:, k * H + m * P:k * H + m * P + P],
                        h0T_h[:, k * NB:(k + 1) * NB],
                        start=(k == 0), stop=(k == 3))
                nc.vector.tensor_copy(qprojT_sb[:, m * NB:(m + 1) * NB], ps[:])
            nc.vector.tensor_copy(c0T_h[:], c0T_sb[:])

        # ---------- A1: keyW projection -> energy (fp16) ----------
        ea = ctx.enter_context(tc.tile_pool(name="energy", bufs=1))
        energy_all = ea.tile([P, 4 * NB * S], F16)
        with tc.tile_pool(name="kw", bufs=1) as kwp, \
             tc.tile_pool(name="ps_pk", bufs=1, space="PSUM") as ps_pk:
            keyWT_sb = kwp.tile([P, 8 * H], F16)
            _load_chunked(nc, keyWT_sb, keyWT_d, 8, H)
            encT_sb = kwp.tile([P, 8 * NB * S], F16)
            for k in range(8):
                sl = slice(k * NB * S, (k + 1) * NB * S)
                nc.sync.dma_start(out=encT_sb[:, sl], in_=encT_d[:, sl])
            pk_tiles = [ps_pk.tile([P, 2 * S], FP32, tag=f"pk{i}",
                                   name=f"pk{i}") for i in range(8)]
            # batches in 2 groups of 8; psum sets alternate so one group's
            # activations overlap the next group's matmuls
            for m in range(4):
                for bg in range(2):
                    pset = pk_tiles[((2 * m + bg) % 2) * 4:
                                    ((2 * m + bg) % 2) * 4 + 4]
                    for k in range(8):
                        stat = keyWT_sb[:, k * H + m * P:k * H + m * P + P]
                        for bpi in range(4):
                            bp = bg * 4 + bpi
                            nc.tensor.matmul(
                                pset[bpi][:], stat,
                                encT_sb[:, (k * NB + 2 * bp) * S:
                                        (k * NB + 2 * bp + 2) * S],
                                start=(k == 0), stop=(k == 7))
                    for bi in range(8):
                        b = bg * 8 + bi
                        nc.scalar.activation(
                            energy_all[:, (m * NB + b) * S:
                                       (m * NB + b + 1) * S],
                            pset[bi // 2][:, (bi % 2) * S:(bi % 2 + 1) * S],
                            AF.Tanh,
                            bias=qprojT_sb[:, m * NB + b:m * NB + b + 1])
        if debug:
            nc.sync.dma_start(out=dbg["d_energy"][:], in_=energy_all[:])

        # phase-B weights: pool opened now that the A1 tiles are freed
        bw = ctx.enter_context(tc.tile_pool(name="bw", bufs=1))
        wih8_sb = bw.tile([P, 2 * FOURH], F8)
        _load_chunked(nc, wih8_sb, wih8_d, 2, FOURH)
        whh_sb = bw.tile([P, 4 * FOURH], F16)
        _load_chunked(nc, whh_sb, whhT_d, 4, FOURH)
        preW8_sb = bw.tile([P, 2 * H], F8)
        _load_chunked(nc, preW8_sb, preW8_d, 2, H)
        preWTh_sb = bw.tile([P, 4 * H], F16)
        _load_chunked(nc, preWTh_sb, preWTh_d, 4, H)
        pwc_sb = bw.tile([P, 8 * H], F16)
        _load_chunked(nc, pwc_sb, preWTc_d, 8, H)
        ocb_sb = bw.tile([P, NB * H], F16)

        # ---------- A2+A3: scores [16,S] + softmax ----------
        with tc.tile_pool(name="smx", bufs=1) as smx, \
             tc.tile_pool(name="ps_sc", bufs=1, space="PSUM") as ps_sc, \
             tc.tile_pool(name="ps_tp", bufs=2, space="PSUM") as ps_tp:
            scps = ps_sc.tile([NB, S], FP32)
            for m in range(4):
                for b in range(NB):
                    nc.tensor.matmul(
                        scps[:],
                        eWd_sb[:, (m * NB + b) * NB:(m * NB + b + 1) * NB],
                        energy_all[:, (m * NB + b) * S:(m * NB + b + 1) * S],
                        start=(m == 0 and b == 0), stop=(m == 3 and b == 15))
            sm = smx.tile([NB, S], FP32)
            nc.vector.tensor_tensor(out=sm[:], in0=scps[:], in1=mask_sb[:],
                                    op=OP.mult)
            nc.vector.tensor_tensor(out=sm[:], in0=sm[:], in1=maskoff_sb[:],
                                    op=OP.add)
            nc.vector.tensor_reduce(nmx[:], sm[:], AX.X, OP.max, negate=True)
            eu = smx.tile([NB, S], F16)
            nc.scalar.activation(eu[:], sm[:], AF.Exp, bias=nmx[:, 0:1],
                                 accum_out=zsum[:])
            nc.vector.reciprocal(rz[:], zsum[:])
            nc.vector.tensor_scalar_mul(alphas_n[:], eu[:], rz[:, 0:1])
            if debug:
                nc.sync.dma_start(out=dbg["d_alpha"][:], in_=alphas_n[:])
            # alphasT masked-diagonal expansion [128, 2*S]
            nc.vector.memset(amask[:], 0.0)
            tpa2 = ps_tp.tile([P, 2 * NB], F16, tag="tp")
            for c in range(2):
                nc.tensor.transpose(tpa2[:, c * NB:(c + 1) * NB],
                                    alphas_n[:, c * P:(c + 1) * P],
                                    identity_h[0:NB, 0:NB])
            for c in range(2):
                nc.vector.tensor_copy(_diag_out(amask, c * S),
                                      tpa2[:, c * NB:(c + 1) * NB])

        # ---------- A5: ctx for all batches -> [16, 2H] ----------
        with tc.tile_pool(name="encp", bufs=8) as encp, \
             tc.tile_pool(name="ps_cu", bufs=1, space="PSUM") as ps_cu, \
             tc.tile_pool(name="ps_tp2", bufs=2, space="PSUM") as ps_tp2:
            ctxps = ps_cu.tile([NB, TWOH], FP32)
            for b in range(NB):
                enc_t = encp.tile([P, 2 * TWOH], F16, tag="enc")
                nc.sync.dma_start(
                    out=enc_t[:].rearrange("p (c d) -> p c d", c=2),
                    in_=enc_d[b * S:(b + 1) * S, :].rearrange(
                        "(c p) d -> p c d", p=P))
                for sc in range(2):
                    for nh in range(2):
                        nc.tensor.matmul(
                            ctxps[:, nh * H:(nh + 1) * H],
                            amask[:, sc * S + b * NB:sc * S + (b + 1) * NB],
                            enc_t[:, sc * TWOH + nh * H:
                                  sc * TWOH + (nh + 1) * H],
                            start=(b == 0 and sc == 0),
                            stop=(b == 15 and sc == 1))
            nc.vector.tensor_copy(ctx_bm[:], ctxps[:])
            if debug:
                nc.sync.dma_start(out=dbg["d_ctx"][:], in_=ctx_bm[:])
            # ctxT [128, 8*NB] via one batched transpose+copy
            tpc = ps_tp2.tile([P, 8 * NB], F16, tag="tp")
            for kc in range(8):
                nc.tensor.transpose(tpc[:, kc * NB:(kc + 1) * NB],
                                    ctx_bm[:, kc * P:(kc + 1) * P],
                                    identity_h[0:NB, 0:NB])
            nc.vector.tensor_copy(ctxT_sb[:], tpc[:])

        # ---------- A7/A8: gate consts + out consts (batch-major) ----------
        wcx_sb = bw.tile([P, 8 * FOURH], F16)
        _load_chunked(nc, wcx_sb, wcxT_d, 8, FOURH)
        with tc.tile_pool(name="ps_gc", bufs=1, space="PSUM") as ps_gc, \
             tc.tile_pool(name="ps_oc", bufs=1, space="PSUM") as ps_oc, \
             tc.tile_pool(name="ps_tp3", bufs=2, space="PSUM") as ps_tp3:
            gcps = ps_gc.tile([NB, FOURH], FP32)
            for n in range(4):
                sl = slice(n * H, (n + 1) * H)
                for k in range(4):
                    nc.tensor.matmul(
                        gcps[:, sl], h0T_h[:, k * NB:(k + 1) * NB],
                        whh_sb[:, k * FOURH + n * H:k * FOURH + (n + 1) * H],
                        start=(k == 0), stop=False)
                for kc in range(8):
                    nc.tensor.matmul(
                        gcps[:, sl], ctxT_sb[:, kc * NB:(kc + 1) * NB],
                        wcx_sb[:, kc * FOURH + n * H:kc * FOURH + (n + 1) * H],
                        start=False, stop=False)
                nc.tensor.matmul(gcps[:, sl], ones16[0:1, :],
                                 biasg_sb[0:1, sl], start=False, stop=True)
            nc.vector.tensor_copy(gc_bm[:], gcps[:])
            if debug:
                nc.sync.dma_start(out=dbg["d_gc"][:], in_=gc_bm[:])
            for half in range(2):
                tp = ps_tp3.tile([P, 8 * NB], F16, tag="tp", name=f"tpg{half}")
                for j in range(8):
                    mg = half * 8 + j
                    nc.tensor.transpose(tp[:, j * NB:(j + 1) * NB],
                                        gc_bm[:, mg * P:(mg + 1) * P],
                                        identity_h[0:NB, 0:NB])
                nc.vector.tensor_copy(
                    gcT_sb[:, half * 8 * NB:(half + 1) * 8 * NB], tp[:])
            ocps = ps_oc.tile([NB, H], FP32)
            for kc in range(8):
                nc.tensor.matmul(ocps[:], ctxT_sb[:, kc * NB:(kc + 1) * NB],
                                 pwc_sb[:, kc * H:(kc + 1) * H],
                                 start=(kc == 0), stop=(kc == 7))
            nc.vector.tensor_copy(oc_sb[:], ocps[:])
            if debug:
                nc.sync.dma_start(out=dbg["d_oc"][:], in_=oc_sb[:])
            # broadcast oc rows to all 128 partitions via DRAM bounce
            nc.sync.dma_start(out=oc_dram[:], in_=oc_sb[:])
            nc.sync.dma_start(
                out=ocb_sb[:].rearrange("p (b n) -> p b n", b=NB),
                in_=bass.AP(oc_dram[:].tensor, 0,
                            [[0, P], [H, NB], [1, H]]))

        # ---------- phase B ----------
        # Per ttile (512 tokens = batches b0,b0+1):
        #   gates via fp8 DoubleRow matmuls into [P,1024] psum pairs
        #   (pair A = i,g; pair B = f,o); sigmoid/tanh on Scalar with
        #   per-(gate,batch) bias column from gcT; LSTM combine skewed one
        #   hs behind so no engine stalls on the serial chain.
        wih8_v = wih8_sb[:].rearrange("p (k n) -> p k n", k=2)
        preW8_v = preW8_sb[:].rearrange("p (k n) -> p k n", k=2)
        with tc.tile_pool(name="embTp", bufs=2) as embTp, \
             tc.tile_pool(name="sgp", bufs=3) as sgp, \
             tc.tile_pool(name="hTp", bufs=2) as hTp, \
             tc.tile_pool(name="outp", bufs=3) as outp, \
             tc.tile_pool(name="ps_tpB", bufs=2, space="PSUM") as ps_tpB, \
             tc.tile_pool(name="ps_g", bufs=1, space="PSUM") as ps_g, \
             tc.tile_pool(name="ps_o", bufs=2, space="PSUM") as ps_o:

            def gate_mm(gps, half, hs, embT_v):
                # half 0 -> gates (i, g) in slices 0/1; half 1 -> (f, o)
                gs = (0, 2) if half == 0 else (1, 3)
                for sl, g in enumerate(gs):
                    mg = g * 4 + hs
                    nc.tensor.matmul(
                        gps[:, sl * H:(sl + 1) * H],
                        wih8_v[:, :, mg * P:(mg + 1) * P], embT_v,
                        start=True, stop=True, perf_mode=PM.DoubleRow)

            def emit_outproj(tt, embT_v, hT_t):
                # out = emb(fp8 DR) + h(fp16) in PSUM; oc added on DVE
                b0 = 2 * tt
                for tci in range(4):
                    b = b0 + tci // 2
                    po = ps_o.tile([P, H], FP32, tag="po", name="po")
                    nc.tensor.matmul(
                        po[:], embT_v[:, :, tci * P:(tci + 1) * P], preW8_v,
                        start=True, stop=False, perf_mode=PM.DoubleRow)
                    for k in range(4):
                        nc.tensor.matmul(
                            po[:],
                            hT_t[:, k * H + tci * P:k * H + tci * P + P],
                            preWTh_sb[:, k * H:(k + 1) * H],
                            start=False, stop=(k == 3))
                    o_t = outp.tile([P, H], F16, tag="o", name="o_t")
                    nc.vector.tensor_tensor(
                        out=o_t[:], in0=po[:],
                        in1=ocb_sb[:, b * H:(b + 1) * H], op=OP.add)
                    nc.sync.dma_start(
                        out=out_d[tt * 512 + tci * P:tt * 512 + (tci + 1) * P,
                                  :],
                        in_=o_t[:])

            def gate_act(gps, half, hs, b0, out_t):
                gs = (0, 2) if half == 0 else (1, 3)
                for sl, g in enumerate(gs):
                    mg = g * 4 + hs
                    fn = AF.Tanh if g == 2 else AF.Sigmoid
                    for x in range(2):
                        cs = slice(sl * H + x * S, sl * H + (x + 1) * S)
                        nc.scalar.activation(
                            out_t[:, cs], gps[:, cs], fn,
                            bias=gcT_sb[:, mg * NB + b0 + x:
                                        mg * NB + b0 + x + 1])

            prevB = None
            for tt in range(NTT):
                b0 = 2 * tt
                # embT (fp8): [128, 2*512] chunk-major
                embT = embTp.tile([P, TWOH], F8, tag="embT")
                for j in range(4):
                    for e in range(2):
                        tp = ps_tpB.tile([P, 2 * P], F8, tag="tpB")
                        tpa = tp[:]
                        tp2 = bass.AP(tpa.tensor, tpa.offset,
                                      [tpa.ap[0], [2, P]])
                        nc.tensor.transpose(
                            tp2,
                            ge_all[:, (tt * 4 + j) * E + e * P:
                                   (tt * 4 + j) * E + (e + 1) * P],
                            identity_8[:])
                        nc.vector.tensor_copy(
                            embT[:, e * H + j * P:e * H + (j + 1) * P], tp2)
                if debug:
                    nc.sync.dma_start(
                        out=dbg["d_embT"][:, tt * TWOH:(tt + 1) * TWOH],
                        in_=embT[:])
                embT_v = embT[:].rearrange("p (k t) -> p k t", k=2)
                hT_t = hTp.tile([P, 4 * H], F16, tag="hT")

                stage = {}   # hs -> (sIG, sFO, t1, t2)
                def combine_tail(hs):
                    sIG, sFO, t1, t2 = stage.pop(hs)
                    cc = sgp.tile([P, H], F16, tag="cc", name="cc")
                    nc.vector.tensor_tensor(out=cc[:], in0=t1[:], in1=t2[:],
                                            op=OP.add)
                    tanc = sgp.tile([P, H], F16, tag="tanc", name="tanc")
                    nc.scalar.activation(tanc[:], cc[:], AF.Tanh)
                    nc.vector.tensor_tensor(
                        out=hT_t[:, hs * H:(hs + 1) * H],
                        in0=sFO[:, H:2 * H], in1=tanc[:], op=OP.mult)

                for hs in range(4):
                    gpsA = ps_g.tile([P, TWOH], FP32, tag="gA", name="gA")
                    gate_mm(gpsA, 0, hs, embT_v)
                    gpsB = ps_g.tile([P, TWOH], FP32, tag="gB", name="gB")
                    gate_mm(gpsB, 1, hs, embT_v)
                    sIG = sgp.tile([P, TWOH], F16, tag="sIG", name="sIG")
                    gate_act(gpsA, 0, hs, b0, sIG)
                    sFO = sgp.tile([P, TWOH], F16, tag="sFO", name="sFO")
                    gate_act(gpsB, 1, hs, b0, sFO)
                    t1 = sgp.tile([P, H], F16, tag="t1", name="t1")
                    nc.vector.tensor_tensor(out=t1[:], in0=sIG[:, 0:H],
                                            in1=sIG[:, H:TWOH], op=OP.mult)
                    t2 = sgp.tile([P, H], F16, tag="t2", name="t2")
                    nc.vector.tensor_tensor(
                        out=t2[:], in0=sFO[:, 0:H],
                        in1=_colpair(c0T_h, hs * NB + b0, S), op=OP.mult)
                    stage[hs] = (sIG, sFO, t1, t2)
                    if hs > 0:
                        combine_tail(hs - 1)
                combine_tail(3)
                if debug:
                    nc.sync.dma_start(
                        out=dbg["d_hT"][:, tt * FOURH:(tt + 1) * FOURH],
                        in_=hT_t[:])
                # output projection for the PREVIOUS ttile (skewed so PE
                # isn't blocked waiting for this ttile's hT)
                if tt > 0:
                    emit_outproj(tt - 1, prevB[0], prevB[1])
                prevB = (embT_v, hT_t)
            emit_outproj(NTT - 1, prevB[0], prevB[1])
    return nc


# ---------------------------------------------------------------------------
# host side
# ---------------------------------------------------------------------------

def _chunkmajor(v, chunks, dtype=np.float32):
    return np.ascontiguousarray(v.reshape(chunks, P).T).astype(dtype)


def prep_inputs(inputs, n_cores=N_CORES):
    f32 = lambda x: np.asarray(x, dtype=np.float32)
    f16 = lambda x: np.ascontiguousarray(
        np.asarray(x, dtype=np.float32)).astype(np.float16)
    f8 = lambda x: np.ascontiguousarray(
        np.asarray(x, dtype=np.float32)).astype(ml_dtypes.float8_e4m3fn)
    tgt_seq = np.asarray(inputs["tgt_seq"]).astype(np.int32)
    enc = f32(inputs["encoder_output"])
    eh = f32(inputs["encoder_hidden"])[0]
    ec = f32(inputs["encoder_cell"])[0]
    src_pos = np.asarray(inputs["src_pos"])
    W_ih = f32(inputs["W_ih"])
    pre_W = f32(inputs["pre_W"])
    eW = f32(inputs["energy_W"])[0]

    eWd = np.zeros((P, 4, NB, NB), np.float16)
    for m in range(4):
        blk = eW[m * P:(m + 1) * P].astype(np.float16)
        for b in range(NB):
            eWd[:, m, b, b] = blk
    eWd = np.ascontiguousarray(eWd.reshape(P, 4 * NB * NB))

    shared = dict(
        emb=f8(inputs["emb"]),
        keyWT=f16(f32(inputs["key_W"]).T),
        queryWT=f16(f32(inputs["query_W"]).T),
        eWd=eWd,
        wih8=f8(W_ih[:, :E].T),
        whhT=f16(f32(inputs["W_hh"]).T),
        wcxT=f16(W_ih[:, E:].T),
        biasg=f16((f32(inputs["b_ih"]) + f32(inputs["b_hh"]))[None, :]),
        bhWT=f16(f32(inputs["bridge_hW"]).T),
        bcWT=f16(f32(inputs["bridge_cW"]).T),
        hb=_chunkmajor(f32(inputs["bridge_hb"]), 4),
        cb=_chunkmajor(f32(inputs["bridge_cb"]), 4),
        preW8=f8(pre_W[:, :E].T),
        preWTh=f16(pre_W[:, E:E + H].T),
        preWTc=f16(pre_W[:, E + H:].T),
    )

    in_maps = []
    for i in range(n_cores):
        sl = slice(i * NB, (i + 1) * NB)
        m = src_pos[sl, 0, :].astype(np.float32)
        enc16 = enc[sl].astype(np.float16)          # [NB, S, 2H]
        # k-major encT: [p, (k b s)]
        ekm = np.ascontiguousarray(
            enc16.transpose(2, 0, 1)                 # [2H, NB, S]
            .reshape(8, P, NB, S).transpose(1, 0, 2, 3)
            .reshape(P, 8 * NB * S))
        in_maps.append(dict(
            encTkm=ekm,
            enc=np.ascontiguousarray(enc16.reshape(NB * S, TWOH)),
            ehT=f16(eh[sl].T),
            ecT=f16(ec[sl].T),
            idx=np.ascontiguousarray(tgt_seq[sl].reshape(-1, P).T),
            mask=np.ascontiguousarray(m),
            maskoff=np.ascontiguousarray(-1e9 * (1.0 - m)),
            **shared,
        ))
    return in_maps, NB


_CACHED = {}


def _get_nc(key=0, debug=False):
    if key not in _CACHED:
        nc = bacc.Bacc("TRN2", target_bir_lowering=False, debug=False)
        build_kernel(nc, debug=debug)
        nc.compile()
        _CACHED[key] = nc
    return _CACHED[key]


def kernel(**inputs):
    in_maps, _ = prep_inputs(inputs, N_CORES)
    nc = _get_nc()
    res = run_bass_kernel_spmd(nc, in_maps, list(range(N_CORES)))
    B = np.asarray(inputs["tgt_seq"]).shape[0]
    out = np.empty((B, T, H), dtype=np.float32)
    for i in range(N_CORES):
        out[i * NB:(i + 1) * NB] = res.results[i]["out"].reshape(NB, T, H)
    return out
